# revision 14
# baseline (speedup 1.0000x reference)
"""Trainium2 Bass kernel for nn_ActionModule (sparse attention, 8 cores).

Sharding: data-parallel over spatial axis S (1560 = 8 x 195 per core).
Each core processes (T=16 frames x 195 spatial) = 3120 tokens through both
branches; small weights replicated; attention is over T=16 frames only.

Per-core tiling: 25 row-tiles of 128 rows; tile = 16 frames x 8 spatial
positions (row p = t*8 + u). The last tile overlaps the previous one so all
tiles are full 128 rows (overlap rows compute identical values).

v2: fully fused per-tile pipeline (no DRAM round-trip for h or xu), all six
big GEMMs in fp8e4m3 DoubleRow perf mode (2 K-chunks per PE pass), softmax
scale folded into the rope-cos tables, zero biases elided, two SBUF-only
vector ops offloaded to the Pool engine.

Phases (per core):
  P0: keyboard cond MLP -> windows -> k2/v2 (tiny, frame-major, bf16)
  per tile: [x|mouse windows] -> MLP -> LN -> qkv -> RMS+rope -> local
  attention (T=16) -> proj+residual -> key_q -> RMS+rope -> cross attention
  vs k2/v2 -> proj+residual -> out (f32, DRAM)
"""
import sys
sys.path.insert(0, '/opt/trn_rl_repo')

import numpy as np
import ml_dtypes

import bass_rust
import concourse.bass as bass
import concourse.bacc as bacc

# Prefer activation-table set 6 (Ln+Exp+Square together): hide Exp/Ln from
# other sets so the table-load insertion pass never thrashes between the
# exp-only and ln-only tables. The ids passed to walrus stay valid (set 6
# really contains all three); we only constrain the chooser.
_orig_get_tables = bacc.get_activation_tables

def _patched_tables(arch):
    tabs = dict(_orig_get_tables(arch))
    out = {}
    for i, (name, funcs) in enumerate(tabs.items()):
        if name != "natural_log_exp_and_others":
            funcs = {f for f in funcs
                     if f not in (mybir.ActivationFunctionType.Exp,
                                  mybir.ActivationFunctionType.Ln)}
        out[name] = set(funcs)
    return out

bacc.get_activation_tables = _patched_tables
import concourse.bacc as _b2
_b2.get_activation_tables = _patched_tables
import concourse.tile as tile
import concourse.mybir as mybir
from concourse.bass_utils import run_bass_kernel_spmd

f32 = mybir.dt.float32
f32r = mybir.dt.float32r
bf16 = mybir.dt.bfloat16
fp8 = mybir.dt.float8e4
DR = mybir.MatmulPerfMode.DoubleRow
Alu = mybir.AluOpType
Act = mybir.ActivationFunctionType
AxX = mybir.AxisListType.X

np_bf16 = ml_dtypes.bfloat16
np_fp8 = ml_dtypes.float8_e4m3

# dims (hardcoded per spec)
T, TH, TW = 16, 30, 52
S = TH * TW            # 1560
SPC = S // 8           # 195 per core
C = 1536               # IMG
CM = 1024
CK = 1024
HID = 128
HD = 64
H = 16                 # heads
PAD_T = 12             # RATIO*WIN
NF = 61
LOCAL = 6
THETA = 256.0
SCALE = 1.0 / 8.0      # 1/sqrt(64), folded into cgq / cgq2

N_TILES = 25
SCH = 8                # spatial positions per tile
ROWS = T * SCH         # 128
S_STARTS = [min(j * SCH, SPC - SCH) for j in range(N_TILES)]


def _rope_tables():
    t = np.arange(T, dtype=np.float32)
    freqs = 1.0 / (THETA ** (np.arange(0, 8, 2, dtype=np.float32) / 8.0))
    ang = t[:, None] * freqs[None, :]
    cos = np.concatenate([np.cos(ang), np.ones((T, 28), np.float32)], axis=1)
    sin = np.concatenate([np.sin(ang), np.zeros((T, 28), np.float32)], axis=1)
    c_exp = np.repeat(cos, 2, axis=1)   # (16, 64): cos[t, d//2]
    s_exp = np.repeat(sin, 2, axis=1)
    return c_exp, s_exp


def _cg(gain, frame_of_row, scale=1.0):
    """CG (R,64): scale*gain[d]*cos[t(p),d] (shared across heads)."""
    c_exp, _ = _rope_tables()
    return np.ascontiguousarray(
        (scale * gain[None, :] * c_exp[frame_of_row]).astype(np.float32))


def _tan(frame_of_row):
    """tan tables: rope correction on the CG-multiplied values; gains cancel:
      tmp[2i]   = -tan(ang_i) * qcg[2i+1],  tmp[2i+1] = tan(ang_i) * qcg[2i]
    """
    c_exp, s_exp = _rope_tables()
    tg = np.zeros((len(frame_of_row), 8), np.float32)
    for i in range(4):
        tn = s_exp[frame_of_row, 2 * i] / c_exp[frame_of_row, 2 * i]
        tg[:, 2 * i] = -tn
        tg[:, 2 * i + 1] = tn
    return np.tile(tg, (1, 8)).astype(np_bf16)  # (R, 64)


def _pair_w(w, n_out):
    """W [K, N] (K mult of 256) -> fp8 paired [128, K//256, 2, N]."""
    K = w.shape[0]
    assert K % 256 == 0 and w.shape[1] == n_out
    wq = np.clip(w, -240, 240).astype(np_fp8)
    return np.ascontiguousarray(
        wq.reshape(K // 256, 2, 128, n_out).transpose(2, 0, 1, 3))


def _build_consts(inp):
    c = {}
    frame_of_row = (np.arange(ROWS) // SCH).astype(np.int64)
    fr16 = np.arange(T, dtype=np.int64)

    # fold softmax scale into the q-side cos-gain tables
    c["cgq"] = _cg(np.asarray(inp["mq_norm_w"], np.float32), frame_of_row, SCALE)
    c["cgk"] = _cg(np.asarray(inp["mk_norm_w"], np.float32), frame_of_row)
    c["cgq2"] = _cg(np.asarray(inp["kq_norm_w"], np.float32), frame_of_row, SCALE)
    c["cg16"] = _cg(np.asarray(inp["kk_norm_w"], np.float32), fr16)
    c["tanx"] = _tan(frame_of_row)
    c["tanx16"] = _tan(fr16)

    t_p = frame_of_row
    u_p = np.arange(ROWS) % SCH
    same_s = u_p[:, None] == u_p[None, :]
    near_t = np.abs(t_p[:, None] - t_p[None, :]) <= LOCAL
    c["madd"] = np.where(same_s & near_t, 0.0, -1e9).astype(np.float32)
    near2 = np.abs(t_p[:, None] - fr16[None, :]) <= LOCAL
    c["madd2"] = np.where(near2, 0.0, -1e9).astype(np.float32)

    mc = np.asarray(inp["mouse_condition"], np.float32)[0]  # (61, 2)
    gm24t = np.zeros((PAD_T * 2, ROWS), np.float32)
    for w in range(PAD_T):
        src = np.maximum(4 * frame_of_row + w - PAD_T, 0)
        gm24t[2 * w] = mc[src, 0]
        gm24t[2 * w + 1] = mc[src, 1]
    c["gm24t"] = gm24t

    # LN gain fold into qkv weights: W' = diag(g) @ W
    g = np.asarray(inp["mm_ln_g"], np.float32)
    wqkv = g[:, None] * np.asarray(inp["tqkv_w"], np.float32)
    c["wqkv8"] = _pair_w(wqkv[:, :2 * CM], 2 * CM)        # q,k: fp8 DoubleRow
    c["wqkvv"] = np.ascontiguousarray(                    # v: bf16
        wqkv[:, 2 * CM:].astype(np_bf16)
        .reshape(8, 128, CM).transpose(1, 0, 2))

    w1 = np.asarray(inp["mm_w1"], np.float32)
    c["w1"] = np.ascontiguousarray(
        w1[:1536].astype(np_bf16).reshape(12, 128, CM).transpose(1, 0, 2))
    c["w1b"] = w1[1536:1560].copy()                       # (24, CM) f32
    c["w2"] = np.ascontiguousarray(
        np.asarray(inp["mm_w2"], np.float32).astype(np_bf16)
        .reshape(8, 128, CM).transpose(1, 0, 2))
    c["wpm"] = _pair_w(np.asarray(inp["proj_mouse_w"], np.float32), C)
    c["wkq"] = _pair_w(np.asarray(inp["key_q_w"], np.float32), CK)
    c["wpk"] = _pair_w(np.asarray(inp["proj_keyboard_w"], np.float32), C)
    c["wkkv"] = np.asarray(inp["key_kv_w"], np.float32).astype(np_bf16)
    c["kbw1"] = np.asarray(inp["kb_w1"], np.float32)
    c["kbw2"] = np.asarray(inp["kb_w2"], np.float32)
    c["kb1c"] = np.asarray(inp["kb_b1"], np.float32).reshape(HID, 1)
    c["kb2c"] = np.asarray(inp["kb_b2"], np.float32).reshape(HID, 1)
    c["condt"] = np.ascontiguousarray(
        np.asarray(inp["keyboard_condition"], np.float32)[0].T)

    c["identb"] = np.eye(128, dtype=np.float32).astype(np_bf16)
    c["identf8"] = np.eye(128, dtype=np.float32).astype(np_fp8)

    # biases of the two mouse MLP layers + folded LN bias (zero in practice)
    b1 = np.asarray(inp["mm_b1"], np.float32)
    b2 = np.asarray(inp["mm_b2"], np.float32)
    bln = np.asarray(inp["mm_ln_b"], np.float32)
    bq = bln @ np.asarray(inp["tqkv_w"], np.float32)
    c["b1r"] = b1.reshape(1, CM)
    c["b2r"] = b2.reshape(1, CM)
    c["bqkv"] = bq.reshape(1, 3 * CM)
    c["ones"] = np.ones((1, 128), np.float32)
    bias_zero = not (np.any(b1) or np.any(b2) or np.any(bq))
    return c, bias_zero


CONST_SPECS = [
    ("cgq", (ROWS, HD), f32),
    ("cgk", (ROWS, HD), f32),
    ("cgq2", (ROWS, HD), f32),
    ("cg16", (T, HD), f32),
    ("tanx", (ROWS, HD), bf16), ("tanx16", (T, HD), bf16),
    ("madd", (ROWS, ROWS), f32), ("madd2", (ROWS, T), f32),
    ("gm24t", (24, ROWS), f32),
    ("wqkv8", (128, 4, 2, 2 * CM), fp8),
    ("wqkvv", (128, 8, CM), bf16),
    ("w1", (128, 12, CM), bf16), ("w1b", (24, CM), f32),
    ("w2", (128, 8, CM), bf16),
    ("wpm", (128, 4, 2, C), fp8),
    ("wkq", (128, 6, 2, CK), fp8),
    ("wpk", (128, 4, 2, C), fp8),
    ("wkkv", (C, 2 * CK), bf16),
    ("kbw1", (6, HID), f32), ("kbw2", (HID, HID), f32),
    ("kb1c", (HID, 1), f32), ("kb2c", (HID, 1), f32),
    ("condt", (6, NF), f32),
    ("identb", (128, 128), bf16), ("identf8", (128, 128), fp8),
    ("b1r", (1, CM), f32), ("b2r", (1, CM), f32),
    ("bqkv", (1, 3 * CM), f32), ("ones", (1, 128), f32),
]
# loaded inside the program body (not persistent pool)
PHASE_WEIGHTS = {"wkkv", "kbw1", "kbw2", "condt"}
BIAS_CONSTS = {"b1r", "b2r", "bqkv", "ones"}


def build_nc(n_tiles=N_TILES, bias_zero=True):
    nc = bacc.Bacc("TRN2", target_bir_lowering=False, debug=False, num_devices=8)
    xt = nc.dram_tensor("xt", [n_tiles * ROWS, C], f32, kind="ExternalInput").ap()
    xtt = nc.dram_tensor("xtt", [n_tiles, 128, 12, 128], bf16,
                         kind="ExternalInput").ap()
    cst = {}
    for name, shp, dt in CONST_SPECS:
        cst[name] = nc.dram_tensor(name, list(shp), dt, kind="ExternalInput").ap()
    out_d = nc.dram_tensor("out", [n_tiles * ROWS, C], f32, kind="ExternalOutput").ap()
    with tile.TileContext(nc) as tc:
        _prog(nc, tc, xt, xtt, cst, out_d, n_tiles, bias_zero)
    nc.compile()
    return nc


def _prog(nc, tc, xt, xtt, cst, out_d, n_tiles, bias_zero):
    from contextlib import ExitStack
    with ExitStack() as ctx:
        pers = ctx.enter_context(tc.tile_pool(name="pers", bufs=1))
        pp_mm = ctx.enter_context(tc.tile_pool(name="ppmm", bufs=3, space="PSUM"))
        pp_tp = ctx.enter_context(tc.tile_pool(name="pptp", bufs=2, space="PSUM"))
        pp_sc = ctx.enter_context(tc.tile_pool(name="ppsc", bufs=3, space="PSUM"))

        # ---- persistent consts + weights ----
        k = {}
        for name, shp, dt in CONST_SPECS:
            if name in PHASE_WEIGHTS or name in BIAS_CONSTS \
                    or name in ("gm24t", "w1b"):
                continue
            t_ = pers.tile(list(shp), dt, tag=name)
            if name in ("wqkv8", "wqkvv", "w1", "w2", "wpm", "wkq", "wpk"):
                for p in range(shp[1]):
                    nc.sync.dma_start(out=t_[:, p], in_=cst[name][:, p])
            else:
                nc.sync.dma_start(out=t_, in_=cst[name])
            k[name] = t_
        gm24t_r = pers.tile([24, ROWS], f32r, tag="gm24t_r")
        nc.sync.dma_start(out=gm24t_r, in_=cst["gm24t"].bitcast(f32r))
        w1b_r = pers.tile([24, CM], f32r, tag="w1b_r")
        nc.sync.dma_start(out=w1b_r, in_=cst["w1b"].bitcast(f32r))
        if not bias_zero:
            for name in BIAS_CONSTS:
                shp = dict((n, s) for n, s, _ in CONST_SPECS)[name]
                t_ = pers.tile(list(shp), f32, tag=name)
                nc.sync.dma_start(out=t_, in_=cst[name])
                k[name] = t_

        eps6 = pers.tile([128, 1], f32, tag="eps6")
        nc.vector.memset(eps6, 1e-6)
        eps5 = pers.tile([128, 1], f32, tag="eps5")
        nc.vector.memset(eps5, 1e-5)

        k2bd = pers.tile([128, 8, 2 * T], bf16, tag="k2bd")
        v2bd = pers.tile([128, 2, 512], bf16, tag="v2bd")

        def trans_copy(src, dst, ident_t, psum_dt, n_chunks, out_view=None):
            """transpose 128-col chunks of src into dst[:, kk, :] (cast on copy)"""
            for kk in range(n_chunks):
                ps = pp_tp.tile([128, 128], psum_dt, tag="tp")
                nc.tensor.transpose(ps, src[:, kk * 128:(kk + 1) * 128], ident_t)
                d = dst[:, kk, :] if out_view is None else out_view(dst, kk)
                nc.vector.tensor_copy(out=d, in_=ps)

        def rms_stage1(scr, q_ps, cg, tg):
            """sq + CG-mult; the only psum readers. Returns (sq, qn, P)."""
            P = q_ps.shape[0]
            HH = 8
            sq = scr.tile([P, 512], bf16, tag="sq" + tg)
            nc.scalar.square(out=sq, in_=q_ps)
            qn = scr.tile([P, 512], bf16, tag="qn" + tg)
            nc.vector.scalar_tensor_tensor(
                out=qn.rearrange("p (h d) -> p h d", h=HH),
                in0=q_ps.rearrange("p (h d) -> p h d", h=HH), scalar=1.0,
                in1=cg.rearrange("p (o d) -> p o d", o=1).broadcast_to([P, HH, HD]),
                op0=Alu.mult, op1=Alu.mult)
            return sq, qn, P

        def rms_stage2(scr, st, tanx, out_half, tg):
            """reduce -> rsqrt -> tan-rope on qn -> apply rstd."""
            sq, qn, P = st
            HH = 8
            qn3 = qn.rearrange("p (h d) -> p h d", h=HH)
            ss = scr.tile([P, HH], f32, tag="ss" + tg)
            nc.vector.tensor_reduce(out=ss, in_=sq.rearrange("p (h d) -> p h d", h=HH),
                                    axis=AxX, op=Alu.add)
            rt = scr.tile([P, HH], f32, tag="rt" + tg)
            nc.scalar.activation(out=rt, in_=ss, func=Act.Ln,
                                 bias=eps6[:P], scale=1.0 / HD)
            rq = scr.tile([P, HH], f32, tag="rq" + tg)
            nc.scalar.activation(out=rq, in_=rt, func=Act.Exp, scale=-0.5)
            # rope correction (first 8 dims/head): tmp = swap(qn) * tanx
            # (even/odd strided 3D ops; >3D DVE ops are rejected)
            tmp = scr.tile([P, HH, 8], bf16, tag="tmp" + tg)

            def _ev(ap, off):
                dims = [list(d) for d in ap.ap]
                step = dims[-1][0]
                nd = dims[:-1] + [[2 * step, 4]]
                return bass_rust.AP(tensor=ap.tensor, offset=ap.offset + off * step,
                                    ap=nd)

            q3r = qn3[:, :, 0:8]
            tmp3 = tmp[:, :, 0:8]
            tx = tanx[:P].rearrange("p (h d) -> p h d", h=HH)
            for off in (0, 1):
                nc.vector.scalar_tensor_tensor(
                    out=_ev(tmp3, off), in0=_ev(q3r, 1 - off), scalar=1.0,
                    in1=_ev(tx, off), op0=Alu.mult, op1=Alu.mult)
            nc.vector.tensor_tensor(out=qn3[:, :, 0:8], in0=qn3[:, :, 0:8], in1=tmp,
                                    op=Alu.add)
            nc.vector.tensor_tensor(
                out=out_half.rearrange("p (h d) -> p h d", h=HH),
                in0=qn3,
                in1=rq.rearrange("p (h o) -> p h o", o=1).broadcast_to([P, HH, HD]),
                op=Alu.mult)

        def rms_rope_half(scr, q_ps, cg, tanx, out_half, tg):
            st = rms_stage1(scr, q_ps, cg, tg)
            rms_stage2(scr, st, tanx, out_half, tg)

        def rms_rope(scr, q_ps2, cg, tanx, out_tile, tg=""):
            if not isinstance(q_ps2, (list, tuple)):
                q_ps2 = [q_ps2[:, 0:512], q_ps2[:, 512:1024]]
            for i in range(2):
                rms_rope_half(scr, q_ps2[i], cg, tanx,
                              out_tile[:, i * 512:(i + 1) * 512], tg + str(i))

        # ================= P0: keyboard k2/v2 =================
        with tc.tile_pool(name="p0w", bufs=1) as p0w:
            wkkv_s = p0w.tile([128, 12, 2 * CK], bf16, tag="wkkv")
            for kk in range(12):
                nc.sync.dma_start(out=wkkv_s[:, kk, :],
                                  in_=cst["wkkv"][kk * 128:(kk + 1) * 128, :])
            condt_r = p0w.tile([6, NF], f32, tag="condt_r")
            nc.sync.dma_start(out=condt_r, in_=cst["condt"])
            kbw1_r = p0w.tile([6, HID], f32, tag="kbw1_r")
            nc.sync.dma_start(out=kbw1_r, in_=cst["kbw1"])
            kbw2_r = p0w.tile([HID, HID], f32, tag="kbw2_r")
            nc.sync.dma_start(out=kbw2_r, in_=cst["kbw2"])
            kb1c = p0w.tile([HID, 1], f32, tag="kb1c")
            nc.sync.dma_start(out=kb1c, in_=cst["kb1c"])
            kb2c = p0w.tile([HID, 1], f32, tag="kb2c")
            nc.sync.dma_start(out=kb2c, in_=cst["kb2c"])

            ps0 = pp_mm.tile([HID, NF], f32, tag="mm")
            nc.tensor.matmul(ps0, kbw1_r, condt_r, start=True, stop=True)
            kb1 = p0w.tile([HID, NF], f32, tag="kb1")
            nc.scalar.activation(out=kb1, in_=ps0, func=Act.Silu,
                                 bias=kb1c, scale=1.0)
            ps1 = pp_mm.tile([HID, NF], f32, tag="mm")
            nc.tensor.matmul(ps1, kbw2_r, kb1, start=True, stop=True)
            kb2 = p0w.tile([HID, NF], bf16, tag="kb2")
            nc.vector.tensor_scalar(out=kb2, in0=ps1, scalar1=kb2c, scalar2=None,
                                    op0=Alu.add)
            gkt = p0w.tile([HID, 12, T], bf16, tag="gkt")
            for w in range(12):
                t0 = (12 - w + 3) // 4  # ceil((12-w)/4)
                if t0 > 0:
                    nc.vector.tensor_copy(out=gkt[:, w, 0:t0],
                                          in_=kb2[:, 0:1].broadcast_to([HID, t0]))
                start = 4 * t0 + w - 12
                src = bass_rust.AP(tensor=kb2.tensor, offset=kb2.offset + start,
                                   ap=[list(kb2.ap[0]), [4, T - t0]])
                nc.vector.tensor_copy(out=gkt[:, w, t0:T], in_=src)
            kv_s = p0w.tile([T, 2 * CK], f32, tag="kv_s")
            for n in range(4):
                ps = pp_mm.tile([T, 512], f32, tag="mm")
                for w in range(12):
                    nc.tensor.matmul(ps, gkt[:, w, :],
                                     wkkv_s[:, w, n * 512:(n + 1) * 512],
                                     start=(w == 0), stop=(w == 11))
                nc.vector.tensor_copy(out=kv_s[:, n * 512:(n + 1) * 512], in_=ps)
            k2n = p0w.tile([T, CK], bf16, tag="k2n")
            rms_rope(p0w, kv_s[:, 0:CK], k["cg16"], k["tanx16"], k2n, tg="p0")
            nc.vector.memset(k2bd, 0.0)
            for kk in range(8):
                ps = pp_tp.tile([128, T], bf16, tag="tp")
                nc.tensor.transpose(ps, k2n[:, kk * 128:(kk + 1) * 128],
                                    k["identb"][:T, :T])
                nc.vector.tensor_copy(out=k2bd[0:HD, kk, 0:T], in_=ps[0:HD, :])
                nc.vector.tensor_copy(out=k2bd[HD:128, kk, T:2 * T], in_=ps[HD:128, :])
            nc.vector.memset(v2bd, 0.0)
            v2b_t = p0w.tile([T, CK], bf16, tag="v2b_t")
            nc.vector.tensor_copy(out=v2b_t, in_=kv_s[:, CK:2 * CK])
            for h in range(H):
                g, hh = h // 8, h % 8
                nc.sync.dma_start(
                    out=v2bd[hh * T:(hh + 1) * T, g, hh * HD:(hh + 1) * HD],
                    in_=v2b_t[:, h * HD:(h + 1) * HD])

        # ================= fused per-tile pipeline =================
        with tc.tile_pool(name="fa", bufs=2) as fa, \
             tc.tile_pool(name="fm", bufs=1) as fm, \
             tc.tile_pool(name="fs", bufs=1) as fs, \
             tc.tile_pool(name="fc", bufs=2) as fc, \
             tc.tile_pool(name="fb", bufs=2) as fb:

            def dr_gemm(ps, actT, w_s, sl, start=True, stop=True):
                npairs = actT.shape[1]
                for p in range(npairs):
                    nc.tensor.matmul(ps, actT[:, p], w_s[:, p, :, sl],
                                     start=(start and p == 0),
                                     stop=(stop and p == npairs - 1),
                                     perf_mode=DR)

            for j in range(n_tiles):
                # ---- load ----
                xT = fa.tile([128, 12, 128], bf16, tag="xT")
                nc.sync.dma_start(out=xT, in_=xtt[j])
                x_s = fa.tile([ROWS, C], f32, tag="x_s")
                nc.sync.dma_start(out=x_s, in_=xt[j * ROWS:(j + 1) * ROWS, :])

                # ---- mm1 + gelu -> h1 (bf16) ----
                h1 = fm.tile([ROWS, CM], bf16, tag="h1")
                for n in range(2):
                    sl = slice(n * 512, (n + 1) * 512)
                    ps1 = pp_mm.tile([ROWS, 512], f32, tag="mm")
                    for kk in range(12):
                        nc.tensor.matmul(ps1, xT[:, kk, :], k["w1"][:, kk, sl],
                                         start=(kk == 0), stop=False)
                    nc.tensor.matmul(ps1, gm24t_r, w1b_r[:, sl],
                                     start=False, stop=bias_zero)
                    if not bias_zero:
                        nc.tensor.matmul(ps1, k["ones"].bitcast(f32r),
                                         k["b1r"].bitcast(f32r)[:, sl],
                                         start=False, stop=True)
                    nc.scalar.activation(out=h1[:, sl], in_=ps1,
                                         func=Act.Gelu_apprx_tanh)
                h1T = fm.tile([128, 8, 128], bf16, tag="h1T")
                trans_copy(h1, h1T, k["identb"], bf16, 8)

                # ---- mm2 + LN -> hn (bf16, normalized) ----
                stats = fs.tile([ROWS, 2, 6], f32, tag="stats")
                ps2h = []
                for n in range(2):
                    sl = slice(n * 512, (n + 1) * 512)
                    ps2 = pp_mm.tile([ROWS, 512], f32, tag="mm")
                    for kk in range(8):
                        nc.tensor.matmul(ps2, h1T[:, kk, :], k["w2"][:, kk, sl],
                                         start=(kk == 0), stop=bias_zero and kk == 7)
                    if not bias_zero:
                        nc.tensor.matmul(ps2, k["ones"].bitcast(f32r),
                                         k["b2r"].bitcast(f32r)[:, sl],
                                         start=False, stop=True)
                    nc.vector.bn_stats(out=stats[:, n, :], in_=ps2)
                    ps2h.append(ps2)
                mv = fs.tile([ROWS, 2], f32, tag="mv")
                nc.vector.bn_aggr(out=mv, in_=stats)
                sd = fs.tile([ROWS, 1], f32, tag="sd")
                nc.scalar.activation(out=sd, in_=mv[:, 1:2], func=Act.Ln,
                                     bias=eps5, scale=1.0)
                rstd = fs.tile([ROWS, 1], f32, tag="rstd")
                nc.scalar.activation(out=rstd, in_=sd, func=Act.Exp, scale=-0.5)
                hn = fm.tile([ROWS, CM], bf16, tag="hn")
                for n in range(2):
                    nc.vector.tensor_scalar(
                        out=hn[:, n * 512:(n + 1) * 512], in0=ps2h[n],
                        scalar1=mv[:, 0:1], scalar2=rstd,
                        op0=Alu.subtract, op1=Alu.mult)
                hTb = fm.tile([128, 8, 128], bf16, tag="hTb")
                trans_copy(hn, hTb, k["identb"], bf16, 8)
                hT8 = fm.tile([128, 4, 2, 128], fp8, tag="hT8")
                nc.vector.tensor_copy(
                    out=hT8.rearrange("p a b f -> p (a b f)"),
                    in_=hTb.rearrange("p a f -> p (a f)"))

                # ---- qkv + RMS/rope (q,k via fp8 DoubleRow; v via bf16) ----
                qn = fm.tile([ROWS, CM], bf16, tag="qnb")
                kn = fm.tile([ROWS, CM], bf16, tag="knb")
                v_s = fm.tile([ROWS, CM], bf16, tag="v_s")
                halves = [(0, 0, k["cgq"], qn), (0, 1, k["cgq"], qn),
                          (1, 0, k["cgk"], kn), (1, 1, k["cgk"], kn)]
                sts = []
                for i, (part, n, cg, _o) in enumerate(halves):
                    sl = slice(part * CM + n * 512, part * CM + (n + 1) * 512)
                    ps = pp_mm.tile([ROWS, 512], f32, tag="mm")
                    dr_gemm(ps, hT8, k["wqkv8"], sl, stop=bias_zero)
                    if not bias_zero:
                        nc.tensor.matmul(ps, k["ones"].bitcast(f32r),
                                         k["bqkv"].bitcast(f32r)[:, sl],
                                         start=False, stop=True)
                    sts.append(rms_stage1(fs, ps, cg, "h%d" % i))
                for i, (part, n, _c, outt) in enumerate(halves):
                    rms_stage2(fs, sts[i], k["tanx"],
                               outt[:, n * 512:(n + 1) * 512], "h%d" % i)
                for n in range(2):
                    sl = slice(n * 512, (n + 1) * 512)
                    ps = pp_mm.tile([ROWS, 512], f32, tag="mm")
                    for kk in range(8):
                        nc.tensor.matmul(ps, hTb[:, kk, :], k["wqkvv"][:, kk, sl],
                                         start=(kk == 0), stop=bias_zero and kk == 7)
                    if not bias_zero:
                        nc.tensor.matmul(
                            ps, k["ones"].bitcast(f32r),
                            k["bqkv"].bitcast(f32r)[:, 2 * CM + n * 512:
                                                    2 * CM + (n + 1) * 512],
                            start=False, stop=True)
                    nc.vector.tensor_copy(out=v_s[:, n * 512:(n + 1) * 512], in_=ps)

                qT = fb.tile([128, 8, 128], bf16, tag="qT")
                trans_copy(qn, qT, k["identb"], bf16, 8)
                kT = fb.tile([128, 8, 128], bf16, tag="kT")
                trans_copy(kn, kT, k["identb"], bf16, 8)

                # ---- mouse attention (pipelined head loop, depth 2) ----
                aoT = fc.tile([128, 8, 128], fp8, tag="aoT")
                sc_l, es_l, sum_l, pv_l = {}, {}, {}, {}

                def stage_scores(h):
                    n_sl, p_off = h // 2, (h % 2) * HD
                    sc = pp_sc.tile([ROWS, ROWS], f32, tag="sc")
                    nc.tensor.matmul(sc, qT[p_off:p_off + HD, n_sl, :],
                                     kT[p_off:p_off + HD, n_sl, :],
                                     start=True, stop=True)
                    nc.vector.tensor_tensor(out=sc, in0=sc, in1=k["madd"],
                                            op=Alu.add)
                    sc_l[h] = sc

                def stage_exp(h):
                    e_s = fb.tile([ROWS, ROWS], bf16, tag="e_s")
                    esum = fb.tile([ROWS, 1], f32, tag="esum")
                    nc.scalar.activation(out=e_s, in_=sc_l.pop(h), func=Act.Exp,
                                         scale=1.0, accum_out=esum)
                    es_l[h], sum_l[h] = e_s, esum

                def stage_out(h):
                    erec = fb.tile([ROWS, 1], f32, tag="erec")
                    nc.vector.reciprocal(out=erec, in_=sum_l.pop(h))
                    e_c = fb.tile([ROWS, ROWS], bf16, tag="e_c")
                    nc.gpsimd.tensor_scalar(out=e_c, in0=es_l.pop(h),
                                            scalar1=erec, scalar2=None,
                                            op0=Alu.mult)
                    pt_ps = pp_tp.tile([128, 128], bf16, tag="tp")
                    nc.tensor.transpose(pt_ps, e_c, k["identb"])
                    pt_s = fb.tile([128, 128], bf16, tag="pt_s")
                    nc.vector.tensor_copy(out=pt_s, in_=pt_ps)
                    if h % 2 == 0:
                        pv_l[h // 2] = pp_sc.tile([128, 128], f32, tag="sc",
                                                  name="pv%d" % (h // 2))
                    pv = pv_l[h // 2]
                    p_off = (h % 2) * HD
                    nc.tensor.matmul(pv[p_off:p_off + HD, :],
                                     v_s[:, h * HD:(h + 1) * HD], pt_s,
                                     start=True, stop=True)
                    if h % 2 == 1:
                        nc.vector.tensor_copy(out=aoT[:, h // 2, :],
                                              in_=pv_l.pop(h // 2))

                for h in range(H + 2):
                    if h < H:
                        stage_scores(h)
                    if 1 <= h <= H:
                        stage_exp(h - 1)
                    if h >= 2:
                        stage_out(h - 2)

                # ---- proj_mouse + residual -> xu (SBUF) ----
                xu_s = fa.tile([ROWS, C], bf16, tag="xu_s")
                aoTp = aoT.rearrange("p (a b) f -> p a b f", b=2)
                for n in range(3):
                    sl = slice(n * 512, (n + 1) * 512)
                    psp = pp_mm.tile([ROWS, 512], f32, tag="mm")
                    dr_gemm(psp, aoTp, k["wpm"], sl)
                    nc.vector.tensor_tensor(out=xu_s[:, sl], in0=psp,
                                            in1=x_s[:, sl], op=Alu.add)

                # ---- key_q + RMS/rope ----
                xuT = fc.tile([128, 6, 2, 128], fp8, tag="xuT")
                trans_copy(xu_s, xuT, k["identb"], bf16, 12,
                           out_view=lambda d, kk: d[:, kk // 2, kk % 2, :])
                q2n = fm.tile([ROWS, CK], bf16, tag="q2n")
                sts2 = []
                for n in range(2):
                    sl = slice(n * 512, (n + 1) * 512)
                    q2_ps = pp_mm.tile([ROWS, 512], f32, tag="mm")
                    dr_gemm(q2_ps, xuT, k["wkq"], sl)
                    sts2.append(rms_stage1(fs, q2_ps, k["cgq2"], "q2%d" % n))
                for n in range(2):
                    rms_stage2(fs, sts2[n], k["tanx"],
                               q2n[:, n * 512:(n + 1) * 512], "q2%d" % n)
                q2T = fm.tile([128, 8, 128], bf16, tag="q2T")
                trans_copy(q2n, q2T, k["identb"], bf16, 8)

                # ---- keyboard attention ----
                sm2 = fb.tile([ROWS, H, T], f32, tag="sm2")
                for pr in range(H // 2):
                    sc2 = pp_sc.tile([ROWS, 2, T], f32, tag="sc")
                    nc.tensor.matmul(sc2.rearrange("p a t -> p (a t)"),
                                     q2T[:, pr, :], k2bd[:, pr, :],
                                     start=True, stop=True)
                    nc.vector.tensor_tensor(
                        out=sm2[:, 2 * pr:2 * pr + 2, :], in0=sc2,
                        in1=k["madd2"].rearrange("p (o t) -> p o t", o=1)
                            .broadcast_to([ROWS, 2, T]),
                        op=Alu.add)
                e2e = fb.tile([ROWS, H, T], f32, tag="e2e")
                nc.scalar.activation(out=e2e, in_=sm2, func=Act.Exp)
                s2 = fb.tile([ROWS, H], f32, tag="s2")
                nc.vector.tensor_reduce(out=s2, in_=e2e, axis=AxX, op=Alu.add)
                r2 = fb.tile([ROWS, H], f32, tag="r2")
                nc.vector.reciprocal(out=r2, in_=s2)
                p2_t = fb.tile([ROWS, H, T], bf16, tag="p2_t")
                nc.gpsimd.tensor_tensor(
                    out=p2_t, in0=e2e,
                    in1=r2.rearrange("p (h o) -> p h o", o=1).broadcast_to([ROWS, H, T]),
                    op=Alu.mult)

                o2T = fb.tile([128, 4, 2, 128], fp8, tag="o2T")
                for g in range(2):
                    pstk = fb.tile([128, 128], bf16, tag="pstk")
                    for hh in range(4):
                        ptp = pp_tp.tile([2 * T, 128], bf16, tag="tp")
                        nc.tensor.transpose(
                            ptp,
                            p2_t[:, 2 * g * 4 + 2 * hh:2 * g * 4 + 2 * hh + 2, :]
                                .rearrange("p a t -> p (a t)"),
                            k["identb"])
                        nc.vector.tensor_copy(out=pstk[hh * 32:(hh + 1) * 32, :],
                                              in_=ptp)
                    for c2 in range(4):
                        ops = pp_sc.tile([128, 128], f32, tag="sc")
                        nc.tensor.matmul(ops, v2bd[:, g, c2 * 128:(c2 + 1) * 128],
                                         pstk, start=True, stop=True)
                        cc = g * 4 + c2
                        nc.vector.tensor_copy(out=o2T[:, cc // 2, cc % 2, :],
                                              in_=ops)

                # ---- proj_keyboard + residual -> out (reuse x_s storage) ----
                for n in range(3):
                    sl = slice(n * 512, (n + 1) * 512)
                    psp = pp_mm.tile([ROWS, 512], f32, tag="mm")
                    dr_gemm(psp, o2T, k["wpk"], sl)
                    nc.vector.tensor_tensor(out=x_s[:, sl], in0=psp,
                                            in1=xu_s[:, sl], op=Alu.add)
                nc.sync.dma_start(out=out_d[j * ROWS:(j + 1) * ROWS, :], in_=x_s)


_NC_CACHE = {}


def _get_nc(n_tiles=N_TILES, bias_zero=True):
    key = (n_tiles, bias_zero)
    if key not in _NC_CACHE:
        _NC_CACHE[key] = build_nc(n_tiles, bias_zero)
    return _NC_CACHE[key]


def _permute_x(x):
    x3 = np.asarray(x, np.float32).reshape(T, S, C)
    s_idx = np.array([[s0 + u for u in range(SCH)] for s0 in S_STARTS])
    shards = []
    for c in range(8):
        g = x3[:, c * SPC + s_idx, :]          # (T, 25, 8, C)
        g = np.ascontiguousarray(g.transpose(1, 0, 2, 3).reshape(N_TILES * ROWS, C))
        shards.append(g)
    return shards


def _unpermute_out(outs):
    res = np.empty((T, S, C), np.float32)
    j_of_s = np.minimum(np.arange(SPC) // SCH, N_TILES - 1)
    u_of_s = np.arange(SPC) - np.array(S_STARTS)[j_of_s]
    for c in range(8):
        o = np.asarray(outs[c], np.float32).reshape(N_TILES, T, SCH, C)
        res[:, c * SPC:(c + 1) * SPC, :] = o[j_of_s, :, u_of_s, :].transpose(1, 0, 2)
    return res.reshape(1, T * S, C)


def _in_maps(inputs):
    consts, bias_zero = _build_consts(inputs)
    shards = _permute_x(inputs["x"])
    np_dt = {f32: np.float32, bf16: np_bf16, fp8: np_fp8}
    in_maps = []
    for c in range(8):
        # x^T per tile, bf16: [n_tiles, 128, 12, 128]
        sh = shards[c].reshape(N_TILES, ROWS, 12, 128)
        xtt = np.ascontiguousarray(
            sh.astype(np_bf16).transpose(0, 3, 2, 1))
        m = {"xt": shards[c], "xtt": xtt}
        for name, shp, dt in CONST_SPECS:
            m[name] = np.ascontiguousarray(
                np.asarray(consts[name]).astype(np_dt[dt]).reshape(shp))
        in_maps.append(m)
    return in_maps, bias_zero


def run(inputs, trace=False):
    in_maps, bias_zero = _in_maps(inputs)
    nc = _get_nc(bias_zero=bias_zero)
    res = run_bass_kernel_spmd(nc, in_maps, core_ids=list(range(8)),
                               trace=trace)
    return _unpermute_out([r["out"] for r in res.results]), res


def kernel(**inputs):
    out, _ = run(inputs)
    return out


if __name__ == "__main__":
    import time
    t0 = time.time()
    nc = build_nc()
    print("build+compile time:", time.time() - t0)


# revision 16
# speedup vs baseline: 1.2693x; 1.2693x over previous
"""Trainium2 Bass kernel for nn_ActionModule (sparse attention, 8 cores).

Sharding: data-parallel over spatial axis S (1560 = 8 x 195 per core).
Each core processes (T=16 frames x 195 spatial) = 3120 tokens through both
branches; small weights replicated; attention is over T=16 frames only.

Per-core tiling: 25 row-tiles of 128 rows; tile = 16 frames x 8 spatial
positions (row p = t*8 + u). The last tile overlaps the previous one so all
tiles are full 128 rows (overlap rows compute identical values).

v2: fully fused per-tile pipeline (no DRAM round-trip for h or xu), all six
big GEMMs in fp8e4m3 DoubleRow perf mode (2 K-chunks per PE pass), softmax
scale folded into the rope-cos tables, zero biases elided, two SBUF-only
vector ops offloaded to the Pool engine.

Phases (per core):
  P0: keyboard cond MLP -> windows -> k2/v2 (tiny, frame-major, bf16)
  per tile: [x|mouse windows] -> MLP -> LN -> qkv -> RMS+rope -> local
  attention (T=16) -> proj+residual -> key_q -> RMS+rope -> cross attention
  vs k2/v2 -> proj+residual -> out (f32, DRAM)
"""
import sys
sys.path.insert(0, '/opt/trn_rl_repo')

import numpy as np
import ml_dtypes

import bass_rust
import concourse.bass as bass
import concourse.bacc as bacc

# Prefer activation-table set 6 (Ln+Exp+Square together): hide Exp/Ln from
# other sets so the table-load insertion pass never thrashes between the
# exp-only and ln-only tables. The ids passed to walrus stay valid (set 6
# really contains all three); we only constrain the chooser.
_orig_get_tables = bacc.get_activation_tables

def _patched_tables(arch):
    tabs = dict(_orig_get_tables(arch))
    out = {}
    for i, (name, funcs) in enumerate(tabs.items()):
        if name != "natural_log_exp_and_others":
            funcs = {f for f in funcs
                     if f not in (mybir.ActivationFunctionType.Exp,
                                  mybir.ActivationFunctionType.Ln)}
        out[name] = set(funcs)
    return out

bacc.get_activation_tables = _patched_tables
import concourse.bacc as _b2
_b2.get_activation_tables = _patched_tables
import concourse.tile as tile
import concourse.mybir as mybir
from concourse.bass_utils import run_bass_kernel_spmd

f32 = mybir.dt.float32
f32r = mybir.dt.float32r
bf16 = mybir.dt.bfloat16
fp8 = mybir.dt.float8e4
DR = mybir.MatmulPerfMode.DoubleRow
Alu = mybir.AluOpType
Act = mybir.ActivationFunctionType
AxX = mybir.AxisListType.X

np_bf16 = ml_dtypes.bfloat16
np_fp8 = ml_dtypes.float8_e4m3

# dims (hardcoded per spec)
T, TH, TW = 16, 30, 52
S = TH * TW            # 1560
SPC = S // 8           # 195 per core
C = 1536               # IMG
CM = 1024
CK = 1024
HID = 128
HD = 64
H = 16                 # heads
PAD_T = 12             # RATIO*WIN
NF = 61
LOCAL = 6
THETA = 256.0
SCALE = 1.0 / 8.0      # 1/sqrt(64), folded into cgq / cgq2

N_TILES = 25
SCH = 8                # spatial positions per tile
ROWS = T * SCH         # 128
S_STARTS = [min(j * SCH, SPC - SCH) for j in range(N_TILES)]


def _rope_tables():
    t = np.arange(T, dtype=np.float32)
    freqs = 1.0 / (THETA ** (np.arange(0, 8, 2, dtype=np.float32) / 8.0))
    ang = t[:, None] * freqs[None, :]
    cos = np.concatenate([np.cos(ang), np.ones((T, 28), np.float32)], axis=1)
    sin = np.concatenate([np.sin(ang), np.zeros((T, 28), np.float32)], axis=1)
    c_exp = np.repeat(cos, 2, axis=1)   # (16, 64): cos[t, d//2]
    s_exp = np.repeat(sin, 2, axis=1)
    return c_exp, s_exp


def _cg(gain, frame_of_row, scale=1.0):
    """CG (R,64): scale*gain[d]*cos[t(p),d] (shared across heads)."""
    c_exp, _ = _rope_tables()
    return np.ascontiguousarray(
        (scale * gain[None, :] * c_exp[frame_of_row]).astype(np.float32))


def _tan(frame_of_row):
    """tan tables: rope correction on the CG-multiplied values; gains cancel:
      tmp[2i]   = -tan(ang_i) * qcg[2i+1],  tmp[2i+1] = tan(ang_i) * qcg[2i]
    """
    c_exp, s_exp = _rope_tables()
    tg = np.zeros((len(frame_of_row), 8), np.float32)
    for i in range(4):
        tn = s_exp[frame_of_row, 2 * i] / c_exp[frame_of_row, 2 * i]
        tg[:, 2 * i] = -tn
        tg[:, 2 * i + 1] = tn
    return np.tile(tg, (1, 8)).astype(np_bf16)  # (R, 64)


def _pair_w(w, n_out):
    """W [K, N] (K mult of 256) -> fp8 paired [128, K//256, 2, N]."""
    K = w.shape[0]
    assert K % 256 == 0 and w.shape[1] == n_out
    wq = np.clip(w, -240, 240).astype(np_fp8)
    return np.ascontiguousarray(
        wq.reshape(K // 256, 2, 128, n_out).transpose(2, 0, 1, 3))


def _build_consts(inp):
    c = {}
    frame_of_row = (np.arange(ROWS) // SCH).astype(np.int64)
    fr16 = np.arange(T, dtype=np.int64)

    # fold softmax scale into the q-side cos-gain tables
    c["cgq"] = _cg(np.asarray(inp["mq_norm_w"], np.float32), frame_of_row, SCALE)
    c["cgk"] = _cg(np.asarray(inp["mk_norm_w"], np.float32), frame_of_row)
    c["cgq2"] = _cg(np.asarray(inp["kq_norm_w"], np.float32), frame_of_row, SCALE)
    c["cg16"] = _cg(np.asarray(inp["kk_norm_w"], np.float32), fr16)
    c["tanx"] = _tan(frame_of_row)
    c["tanx16"] = _tan(fr16)

    t_p = frame_of_row
    u_p = np.arange(ROWS) % SCH
    same_s = u_p[:, None] == u_p[None, :]
    near_t = np.abs(t_p[:, None] - t_p[None, :]) <= LOCAL
    c["madd"] = np.where(same_s & near_t, 0.0, -1e9).astype(np.float32)
    near2 = np.abs(t_p[:, None] - fr16[None, :]) <= LOCAL
    c["madd2"] = np.where(near2, 0.0, -1e9).astype(np.float32)

    mc = np.asarray(inp["mouse_condition"], np.float32)[0]  # (61, 2)
    gm24t = np.zeros((PAD_T * 2, ROWS), np.float32)
    for w in range(PAD_T):
        src = np.maximum(4 * frame_of_row + w - PAD_T, 0)
        gm24t[2 * w] = mc[src, 0]
        gm24t[2 * w + 1] = mc[src, 1]
    c["gm24t"] = gm24t

    # LN gain fold into qkv weights: W' = diag(g) @ W
    g = np.asarray(inp["mm_ln_g"], np.float32)
    wqkv = g[:, None] * np.asarray(inp["tqkv_w"], np.float32)
    c["wqkv8"] = _pair_w(wqkv[:, :2 * CM], 2 * CM)        # q,k: fp8 DoubleRow
    c["wqkvv"] = np.ascontiguousarray(                    # v: bf16
        wqkv[:, 2 * CM:].astype(np_bf16)
        .reshape(8, 128, CM).transpose(1, 0, 2))

    w1 = np.asarray(inp["mm_w1"], np.float32)
    c["w1"] = np.ascontiguousarray(
        w1[:1536].astype(np_bf16).reshape(12, 128, CM).transpose(1, 0, 2))
    c["w1b"] = w1[1536:1560].copy()                       # (24, CM) f32
    c["w2"] = np.ascontiguousarray(
        np.asarray(inp["mm_w2"], np.float32).astype(np_bf16)
        .reshape(8, 128, CM).transpose(1, 0, 2))
    c["wpm"] = _pair_w(np.asarray(inp["proj_mouse_w"], np.float32), C)
    c["wkq"] = _pair_w(np.asarray(inp["key_q_w"], np.float32), CK)
    c["wpk"] = _pair_w(np.asarray(inp["proj_keyboard_w"], np.float32), C)
    c["wkkv"] = np.asarray(inp["key_kv_w"], np.float32).astype(np_bf16)
    c["kbw1"] = np.asarray(inp["kb_w1"], np.float32)
    c["kbw2"] = np.asarray(inp["kb_w2"], np.float32)
    c["kb1c"] = np.asarray(inp["kb_b1"], np.float32).reshape(HID, 1)
    c["kb2c"] = np.asarray(inp["kb_b2"], np.float32).reshape(HID, 1)
    c["condt"] = np.ascontiguousarray(
        np.asarray(inp["keyboard_condition"], np.float32)[0].T)

    c["identb"] = np.eye(128, dtype=np.float32).astype(np_bf16)
    c["identf8"] = np.eye(128, dtype=np.float32).astype(np_fp8)

    # biases of the two mouse MLP layers + folded LN bias (zero in practice)
    b1 = np.asarray(inp["mm_b1"], np.float32)
    b2 = np.asarray(inp["mm_b2"], np.float32)
    bln = np.asarray(inp["mm_ln_b"], np.float32)
    bq = bln @ np.asarray(inp["tqkv_w"], np.float32)
    c["b1r"] = b1.reshape(1, CM)
    c["b2r"] = b2.reshape(1, CM)
    c["bqkv"] = bq.reshape(1, 3 * CM)
    c["ones"] = np.ones((1, 128), np.float32)
    bias_zero = not (np.any(b1) or np.any(b2) or np.any(bq))
    return c, bias_zero


CONST_SPECS = [
    ("cgq", (ROWS, HD), f32),
    ("cgk", (ROWS, HD), f32),
    ("cgq2", (ROWS, HD), f32),
    ("cg16", (T, HD), f32),
    ("tanx", (ROWS, HD), bf16), ("tanx16", (T, HD), bf16),
    ("madd", (ROWS, ROWS), f32), ("madd2", (ROWS, T), f32),
    ("gm24t", (24, ROWS), f32),
    ("wqkv8", (128, 4, 2, 2 * CM), fp8),
    ("wqkvv", (128, 8, CM), bf16),
    ("w1", (128, 12, CM), bf16), ("w1b", (24, CM), f32),
    ("w2", (128, 8, CM), bf16),
    ("wpm", (128, 4, 2, C), fp8),
    ("wkq", (128, 6, 2, CK), fp8),
    ("wpk", (128, 4, 2, C), fp8),
    ("wkkv", (C, 2 * CK), bf16),
    ("kbw1", (6, HID), f32), ("kbw2", (HID, HID), f32),
    ("kb1c", (HID, 1), f32), ("kb2c", (HID, 1), f32),
    ("condt", (6, NF), f32),
    ("identb", (128, 128), bf16), ("identf8", (128, 128), fp8),
    ("b1r", (1, CM), f32), ("b2r", (1, CM), f32),
    ("bqkv", (1, 3 * CM), f32), ("ones", (1, 128), f32),
]
# loaded inside the program body (not persistent pool)
PHASE_WEIGHTS = {"wkkv", "kbw1", "kbw2", "condt"}
BIAS_CONSTS = {"b1r", "b2r", "bqkv", "ones"}


def build_nc(n_tiles=N_TILES, bias_zero=True):
    nc = bacc.Bacc("TRN2", target_bir_lowering=False, debug=False, num_devices=8)
    xt = nc.dram_tensor("xt", [n_tiles * ROWS, C], f32, kind="ExternalInput").ap()
    xtt = nc.dram_tensor("xtt", [n_tiles, 128, 12, 128], bf16,
                         kind="ExternalInput").ap()
    cst = {}
    for name, shp, dt in CONST_SPECS:
        cst[name] = nc.dram_tensor(name, list(shp), dt, kind="ExternalInput").ap()
    out_d = nc.dram_tensor("out", [n_tiles * ROWS, C], f32, kind="ExternalOutput").ap()
    with tile.TileContext(nc) as tc:
        _prog(nc, tc, xt, xtt, cst, out_d, n_tiles, bias_zero)
    nc.compile()
    return nc


def _prog(nc, tc, xt, xtt, cst, out_d, n_tiles, bias_zero):
    from contextlib import ExitStack
    with ExitStack() as ctx:
        pers = ctx.enter_context(tc.tile_pool(name="pers", bufs=1))
        pp_mm = ctx.enter_context(tc.tile_pool(name="ppmm", bufs=3, space="PSUM"))
        pp_tp = ctx.enter_context(tc.tile_pool(name="pptp", bufs=2, space="PSUM"))
        pp_sc = ctx.enter_context(tc.tile_pool(name="ppsc", bufs=3, space="PSUM"))

        # ---- persistent consts + weights ----
        k = {}
        for name, shp, dt in CONST_SPECS:
            if name in PHASE_WEIGHTS or name in BIAS_CONSTS \
                    or name in ("gm24t", "w1b"):
                continue
            t_ = pers.tile(list(shp), dt, tag=name)
            if name in ("wqkv8", "wqkvv", "w1", "w2", "wpm", "wkq", "wpk"):
                for p in range(shp[1]):
                    nc.sync.dma_start(out=t_[:, p], in_=cst[name][:, p])
            else:
                nc.sync.dma_start(out=t_, in_=cst[name])
            k[name] = t_
        gm24t_r = pers.tile([24, ROWS], f32r, tag="gm24t_r")
        nc.sync.dma_start(out=gm24t_r, in_=cst["gm24t"].bitcast(f32r))
        w1b_r = pers.tile([24, CM], f32r, tag="w1b_r")
        nc.sync.dma_start(out=w1b_r, in_=cst["w1b"].bitcast(f32r))
        if not bias_zero:
            for name in BIAS_CONSTS:
                shp = dict((n, s) for n, s, _ in CONST_SPECS)[name]
                t_ = pers.tile(list(shp), f32, tag=name)
                nc.sync.dma_start(out=t_, in_=cst[name])
                k[name] = t_

        eps6 = pers.tile([128, 1], f32, tag="eps6")
        nc.vector.memset(eps6, 1e-6)
        eps5 = pers.tile([128, 1], f32, tag="eps5")
        nc.vector.memset(eps5, 1e-5)

        k2bd = pers.tile([128, 8, 2 * T], bf16, tag="k2bd")
        v2bd = pers.tile([128, 2, 512], bf16, tag="v2bd")

        def trans_copy(src, dst, ident_t, psum_dt, n_chunks, out_view=None):
            """transpose 128-col chunks of src into dst[:, kk, :] (cast on copy)"""
            for kk in range(n_chunks):
                ps = pp_tp.tile([128, 128], psum_dt, tag="tp")
                nc.tensor.transpose(ps, src[:, kk * 128:(kk + 1) * 128], ident_t)
                d = dst[:, kk, :] if out_view is None else out_view(dst, kk)
                nc.vector.tensor_copy(out=d, in_=ps)

        def rms_stage1(scr, q_ps, cg, tg):
            """sq + CG-mult; the only psum readers. Returns (sq, qn, P)."""
            P = q_ps.shape[0]
            HH = 8
            sq = scr.tile([P, 512], bf16, tag="sq" + tg)
            nc.scalar.square(out=sq, in_=q_ps)
            qn = scr.tile([P, 512], bf16, tag="qn" + tg)
            nc.vector.scalar_tensor_tensor(
                out=qn.rearrange("p (h d) -> p h d", h=HH),
                in0=q_ps.rearrange("p (h d) -> p h d", h=HH), scalar=1.0,
                in1=cg.rearrange("p (o d) -> p o d", o=1).broadcast_to([P, HH, HD]),
                op0=Alu.mult, op1=Alu.mult)
            return sq, qn, P

        def rms_stage2(scr, st, tanx, out_half, tg):
            """reduce -> rsqrt -> tan-rope on qn -> apply rstd."""
            sq, qn, P = st
            HH = 8
            qn3 = qn.rearrange("p (h d) -> p h d", h=HH)
            ss = scr.tile([P, HH], f32, tag="ss" + tg)
            nc.vector.tensor_reduce(out=ss, in_=sq.rearrange("p (h d) -> p h d", h=HH),
                                    axis=AxX, op=Alu.add)
            rt = scr.tile([P, HH], f32, tag="rt" + tg)
            nc.scalar.activation(out=rt, in_=ss, func=Act.Ln,
                                 bias=eps6[:P], scale=1.0 / HD)
            rq = scr.tile([P, HH], f32, tag="rq" + tg)
            nc.scalar.activation(out=rq, in_=rt, func=Act.Exp, scale=-0.5)
            # rope correction (first 8 dims/head): tmp = swap(qn) * tanx
            # (even/odd strided 3D ops; >3D DVE ops are rejected)
            tmp = scr.tile([P, HH, 8], bf16, tag="tmp" + tg)

            def _ev(ap, off):
                dims = [list(d) for d in ap.ap]
                step = dims[-1][0]
                nd = dims[:-1] + [[2 * step, 4]]
                return bass_rust.AP(tensor=ap.tensor, offset=ap.offset + off * step,
                                    ap=nd)

            q3r = qn3[:, :, 0:8]
            tmp3 = tmp[:, :, 0:8]
            tx = tanx[:P].rearrange("p (h d) -> p h d", h=HH)
            for off in (0, 1):
                nc.vector.scalar_tensor_tensor(
                    out=_ev(tmp3, off), in0=_ev(q3r, 1 - off), scalar=1.0,
                    in1=_ev(tx, off), op0=Alu.mult, op1=Alu.mult)
            nc.vector.tensor_tensor(out=qn3[:, :, 0:8], in0=qn3[:, :, 0:8], in1=tmp,
                                    op=Alu.add)
            nc.vector.tensor_tensor(
                out=out_half.rearrange("p (h d) -> p h d", h=HH),
                in0=qn3,
                in1=rq.rearrange("p (h o) -> p h o", o=1).broadcast_to([P, HH, HD]),
                op=Alu.mult)

        def rms_rope_half(scr, q_ps, cg, tanx, out_half, tg):
            st = rms_stage1(scr, q_ps, cg, tg)
            rms_stage2(scr, st, tanx, out_half, tg)

        def rms_rope(scr, q_ps2, cg, tanx, out_tile, tg=""):
            if not isinstance(q_ps2, (list, tuple)):
                q_ps2 = [q_ps2[:, 0:512], q_ps2[:, 512:1024]]
            for i in range(2):
                rms_rope_half(scr, q_ps2[i], cg, tanx,
                              out_tile[:, i * 512:(i + 1) * 512], tg + str(i))

        # ================= P0: keyboard k2/v2 =================
        with tc.tile_pool(name="p0w", bufs=1) as p0w:
            wkkv_s = p0w.tile([128, 12, 2 * CK], bf16, tag="wkkv")
            for kk in range(12):
                nc.sync.dma_start(out=wkkv_s[:, kk, :],
                                  in_=cst["wkkv"][kk * 128:(kk + 1) * 128, :])
            condt_r = p0w.tile([6, NF], f32, tag="condt_r")
            nc.sync.dma_start(out=condt_r, in_=cst["condt"])
            kbw1_r = p0w.tile([6, HID], f32, tag="kbw1_r")
            nc.sync.dma_start(out=kbw1_r, in_=cst["kbw1"])
            kbw2_r = p0w.tile([HID, HID], f32, tag="kbw2_r")
            nc.sync.dma_start(out=kbw2_r, in_=cst["kbw2"])
            kb1c = p0w.tile([HID, 1], f32, tag="kb1c")
            nc.sync.dma_start(out=kb1c, in_=cst["kb1c"])
            kb2c = p0w.tile([HID, 1], f32, tag="kb2c")
            nc.sync.dma_start(out=kb2c, in_=cst["kb2c"])

            ps0 = pp_mm.tile([HID, NF], f32, tag="mm")
            nc.tensor.matmul(ps0, kbw1_r, condt_r, start=True, stop=True)
            kb1 = p0w.tile([HID, NF], f32, tag="kb1")
            nc.scalar.activation(out=kb1, in_=ps0, func=Act.Silu,
                                 bias=kb1c, scale=1.0)
            ps1 = pp_mm.tile([HID, NF], f32, tag="mm")
            nc.tensor.matmul(ps1, kbw2_r, kb1, start=True, stop=True)
            kb2 = p0w.tile([HID, NF], bf16, tag="kb2")
            nc.vector.tensor_scalar(out=kb2, in0=ps1, scalar1=kb2c, scalar2=None,
                                    op0=Alu.add)
            gkt = p0w.tile([HID, 12, T], bf16, tag="gkt")
            for w in range(12):
                t0 = (12 - w + 3) // 4  # ceil((12-w)/4)
                if t0 > 0:
                    nc.vector.tensor_copy(out=gkt[:, w, 0:t0],
                                          in_=kb2[:, 0:1].broadcast_to([HID, t0]))
                start = 4 * t0 + w - 12
                src = bass_rust.AP(tensor=kb2.tensor, offset=kb2.offset + start,
                                   ap=[list(kb2.ap[0]), [4, T - t0]])
                nc.vector.tensor_copy(out=gkt[:, w, t0:T], in_=src)
            kv_s = p0w.tile([T, 2 * CK], f32, tag="kv_s")
            for n in range(4):
                ps = pp_mm.tile([T, 512], f32, tag="mm")
                for w in range(12):
                    nc.tensor.matmul(ps, gkt[:, w, :],
                                     wkkv_s[:, w, n * 512:(n + 1) * 512],
                                     start=(w == 0), stop=(w == 11))
                nc.vector.tensor_copy(out=kv_s[:, n * 512:(n + 1) * 512], in_=ps)
            k2n = p0w.tile([T, CK], bf16, tag="k2n")
            rms_rope(p0w, kv_s[:, 0:CK], k["cg16"], k["tanx16"], k2n, tg="p0")
            nc.vector.memset(k2bd, 0.0)
            for kk in range(8):
                ps = pp_tp.tile([128, T], bf16, tag="tp")
                nc.tensor.transpose(ps, k2n[:, kk * 128:(kk + 1) * 128],
                                    k["identb"][:T, :T])
                nc.vector.tensor_copy(out=k2bd[0:HD, kk, 0:T], in_=ps[0:HD, :])
                nc.vector.tensor_copy(out=k2bd[HD:128, kk, T:2 * T], in_=ps[HD:128, :])
            nc.vector.memset(v2bd, 0.0)
            v2b_t = p0w.tile([T, CK], bf16, tag="v2b_t")
            nc.vector.tensor_copy(out=v2b_t, in_=kv_s[:, CK:2 * CK])
            for h in range(H):
                g, hh = h // 8, h % 8
                nc.sync.dma_start(
                    out=v2bd[hh * T:(hh + 1) * T, g, hh * HD:(hh + 1) * HD],
                    in_=v2b_t[:, h * HD:(h + 1) * HD])

        # ================= fused per-tile pipeline =================
        with tc.tile_pool(name="fa", bufs=2) as fa, \
             tc.tile_pool(name="fm", bufs=1) as fm, \
             tc.tile_pool(name="fs", bufs=1) as fs, \
             tc.tile_pool(name="fc", bufs=2) as fc, \
             tc.tile_pool(name="fb", bufs=2) as fb:

            def dr_gemm(ps, actT, w_s, sl, start=True, stop=True):
                npairs = actT.shape[1]
                for p in range(npairs):
                    nc.tensor.matmul(ps, actT[:, p], w_s[:, p, :, sl],
                                     start=(start and p == 0),
                                     stop=(stop and p == npairs - 1),
                                     perf_mode=DR)

            for j in range(n_tiles):
                # ---- load ----
                xT = fa.tile([128, 12, 128], bf16, tag="xT")
                nc.sync.dma_start(out=xT, in_=xtt[j])
                x_s = fa.tile([ROWS, C], f32, tag="x_s")
                nc.sync.dma_start(out=x_s, in_=xt[j * ROWS:(j + 1) * ROWS, :])

                # ---- mm1 + gelu -> h1 (bf16) ----
                h1 = fm.tile([ROWS, CM], bf16, tag="h1")
                for n in range(2):
                    sl = slice(n * 512, (n + 1) * 512)
                    ps1 = pp_mm.tile([ROWS, 512], f32, tag="mm")
                    for kk in range(12):
                        nc.tensor.matmul(ps1, xT[:, kk, :], k["w1"][:, kk, sl],
                                         start=(kk == 0), stop=False)
                    nc.tensor.matmul(ps1, gm24t_r, w1b_r[:, sl],
                                     start=False, stop=bias_zero)
                    if not bias_zero:
                        nc.tensor.matmul(ps1, k["ones"].bitcast(f32r),
                                         k["b1r"].bitcast(f32r)[:, sl],
                                         start=False, stop=True)
                    nc.scalar.activation(out=h1[:, sl], in_=ps1,
                                         func=Act.Gelu_apprx_tanh)
                h1T = fm.tile([128, 8, 128], bf16, tag="h1T")
                trans_copy(h1, h1T, k["identb"], bf16, 8)

                # ---- mm2 + LN -> hn (bf16, normalized) ----
                stats = fs.tile([ROWS, 2, 6], f32, tag="stats")
                ps2h = []
                for n in range(2):
                    sl = slice(n * 512, (n + 1) * 512)
                    ps2 = pp_mm.tile([ROWS, 512], f32, tag="mm")
                    for kk in range(8):
                        nc.tensor.matmul(ps2, h1T[:, kk, :], k["w2"][:, kk, sl],
                                         start=(kk == 0), stop=bias_zero and kk == 7)
                    if not bias_zero:
                        nc.tensor.matmul(ps2, k["ones"].bitcast(f32r),
                                         k["b2r"].bitcast(f32r)[:, sl],
                                         start=False, stop=True)
                    nc.vector.bn_stats(out=stats[:, n, :], in_=ps2)
                    ps2h.append(ps2)
                mv = fs.tile([ROWS, 2], f32, tag="mv")
                nc.vector.bn_aggr(out=mv, in_=stats)
                sd = fs.tile([ROWS, 1], f32, tag="sd")
                nc.scalar.activation(out=sd, in_=mv[:, 1:2], func=Act.Ln,
                                     bias=eps5, scale=1.0)
                rstd = fs.tile([ROWS, 1], f32, tag="rstd")
                nc.scalar.activation(out=rstd, in_=sd, func=Act.Exp, scale=-0.5)
                hn = fm.tile([ROWS, CM], bf16, tag="hn")
                for n in range(2):
                    nc.vector.tensor_scalar(
                        out=hn[:, n * 512:(n + 1) * 512], in0=ps2h[n],
                        scalar1=mv[:, 0:1], scalar2=rstd,
                        op0=Alu.subtract, op1=Alu.mult)
                hTb = fm.tile([128, 8, 128], bf16, tag="hTb")
                trans_copy(hn, hTb, k["identb"], bf16, 8)
                hT8 = fm.tile([128, 4, 2, 128], fp8, tag="hT8")
                nc.vector.tensor_copy(
                    out=hT8.rearrange("p a b f -> p (a b f)"),
                    in_=hTb.rearrange("p a f -> p (a f)"))

                # ---- qkv + RMS/rope (q,k via fp8 DoubleRow; v via bf16) ----
                qn = fm.tile([ROWS, CM], bf16, tag="qnb")
                kn = fm.tile([ROWS, CM], bf16, tag="knb")
                v_s = fm.tile([ROWS, CM], bf16, tag="v_s")
                halves = [(0, 0, k["cgq"], qn), (0, 1, k["cgq"], qn),
                          (1, 0, k["cgk"], kn), (1, 1, k["cgk"], kn)]
                sts = []
                for i, (part, n, cg, _o) in enumerate(halves):
                    sl = slice(part * CM + n * 512, part * CM + (n + 1) * 512)
                    ps = pp_mm.tile([ROWS, 512], f32, tag="mm")
                    dr_gemm(ps, hT8, k["wqkv8"], sl, stop=bias_zero)
                    if not bias_zero:
                        nc.tensor.matmul(ps, k["ones"].bitcast(f32r),
                                         k["bqkv"].bitcast(f32r)[:, sl],
                                         start=False, stop=True)
                    sts.append(rms_stage1(fs, ps, cg, "h%d" % i))
                for i, (part, n, _c, outt) in enumerate(halves):
                    rms_stage2(fs, sts[i], k["tanx"],
                               outt[:, n * 512:(n + 1) * 512], "h%d" % i)
                for n in range(2):
                    sl = slice(n * 512, (n + 1) * 512)
                    ps = pp_mm.tile([ROWS, 512], f32, tag="mm")
                    for kk in range(8):
                        nc.tensor.matmul(ps, hTb[:, kk, :], k["wqkvv"][:, kk, sl],
                                         start=(kk == 0), stop=bias_zero and kk == 7)
                    if not bias_zero:
                        nc.tensor.matmul(
                            ps, k["ones"].bitcast(f32r),
                            k["bqkv"].bitcast(f32r)[:, 2 * CM + n * 512:
                                                    2 * CM + (n + 1) * 512],
                            start=False, stop=True)
                    nc.vector.tensor_copy(out=v_s[:, n * 512:(n + 1) * 512], in_=ps)

                qT = fb.tile([128, 8, 128], bf16, tag="qT")
                trans_copy(qn, qT, k["identb"], bf16, 8)
                kT = fb.tile([128, 8, 128], bf16, tag="kT")
                trans_copy(kn, kT, k["identb"], bf16, 8)

                # ---- mouse attention (pipelined head loop, depth 2) ----
                aoT = fc.tile([128, 8, 128], fp8, tag="aoT")
                sc_l, es_l, sum_l, pv_l = {}, {}, {}, {}

                def stage_scores(h):
                    n_sl, p_off = h // 2, (h % 2) * HD
                    sc = pp_sc.tile([ROWS, ROWS], f32, tag="sc")
                    nc.tensor.matmul(sc, qT[p_off:p_off + HD, n_sl, :],
                                     kT[p_off:p_off + HD, n_sl, :],
                                     start=True, stop=True)
                    nc.vector.tensor_tensor(out=sc, in0=sc, in1=k["madd"],
                                            op=Alu.add)
                    sc_l[h] = sc

                def stage_exp(h):
                    e_s = fb.tile([ROWS, ROWS], bf16, tag="e_s")
                    esum = fb.tile([ROWS, 1], f32, tag="esum")
                    nc.scalar.activation(out=e_s, in_=sc_l.pop(h), func=Act.Exp,
                                         scale=1.0, accum_out=esum)
                    es_l[h], sum_l[h] = e_s, esum

                def stage_out(h):
                    erec = fb.tile([ROWS, 1], f32, tag="erec")
                    nc.vector.reciprocal(out=erec, in_=sum_l.pop(h))
                    e_c = fb.tile([ROWS, ROWS], bf16, tag="e_c")
                    nc.scalar.activation(out=e_c, in_=es_l.pop(h),
                                         func=Act.Copy, scale=erec)
                    pt_ps = pp_tp.tile([128, 128], bf16, tag="tp")
                    nc.tensor.transpose(pt_ps, e_c, k["identb"])
                    pt_s = fb.tile([128, 128], bf16, tag="pt_s")
                    nc.vector.tensor_copy(out=pt_s, in_=pt_ps)
                    if h % 2 == 0:
                        pv_l[h // 2] = pp_sc.tile([128, 128], f32, tag="sc",
                                                  name="pv%d" % (h // 2))
                    pv = pv_l[h // 2]
                    p_off = (h % 2) * HD
                    nc.tensor.matmul(pv[p_off:p_off + HD, :],
                                     v_s[:, h * HD:(h + 1) * HD], pt_s,
                                     start=True, stop=True)
                    if h % 2 == 1:
                        nc.vector.tensor_copy(out=aoT[:, h // 2, :],
                                              in_=pv_l.pop(h // 2))

                for h in range(H + 2):
                    if h < H:
                        stage_scores(h)
                    if 1 <= h <= H:
                        stage_exp(h - 1)
                    if h >= 2:
                        stage_out(h - 2)

                # ---- proj_mouse + residual -> xu (SBUF) ----
                xu_s = fa.tile([ROWS, C], bf16, tag="xu_s")
                aoTp = aoT.rearrange("p (a b) f -> p a b f", b=2)
                for n in range(3):
                    sl = slice(n * 512, (n + 1) * 512)
                    psp = pp_mm.tile([ROWS, 512], f32, tag="mm")
                    dr_gemm(psp, aoTp, k["wpm"], sl)
                    nc.vector.tensor_tensor(out=xu_s[:, sl], in0=psp,
                                            in1=x_s[:, sl], op=Alu.add)

                # ---- key_q + RMS/rope ----
                xuT = fc.tile([128, 6, 2, 128], fp8, tag="xuT")
                trans_copy(xu_s, xuT, k["identb"], bf16, 12,
                           out_view=lambda d, kk: d[:, kk // 2, kk % 2, :])
                q2n = fm.tile([ROWS, CK], bf16, tag="q2n")
                sts2 = []
                for n in range(2):
                    sl = slice(n * 512, (n + 1) * 512)
                    q2_ps = pp_mm.tile([ROWS, 512], f32, tag="mm")
                    dr_gemm(q2_ps, xuT, k["wkq"], sl)
                    sts2.append(rms_stage1(fs, q2_ps, k["cgq2"], "q2%d" % n))
                for n in range(2):
                    rms_stage2(fs, sts2[n], k["tanx"],
                               q2n[:, n * 512:(n + 1) * 512], "q2%d" % n)
                q2T = fm.tile([128, 8, 128], bf16, tag="q2T")
                trans_copy(q2n, q2T, k["identb"], bf16, 8)

                # ---- keyboard attention ----
                sm2 = fb.tile([ROWS, H, T], f32, tag="sm2")
                for pr in range(H // 2):
                    sc2 = pp_sc.tile([ROWS, 2, T], f32, tag="sc")
                    nc.tensor.matmul(sc2.rearrange("p a t -> p (a t)"),
                                     q2T[:, pr, :], k2bd[:, pr, :],
                                     start=True, stop=True)
                    nc.vector.tensor_tensor(
                        out=sm2[:, 2 * pr:2 * pr + 2, :], in0=sc2,
                        in1=k["madd2"].rearrange("p (o t) -> p o t", o=1)
                            .broadcast_to([ROWS, 2, T]),
                        op=Alu.add)
                e2e = fb.tile([ROWS, H, T], f32, tag="e2e")
                nc.scalar.activation(out=e2e, in_=sm2, func=Act.Exp)
                s2 = fb.tile([ROWS, H], f32, tag="s2")
                nc.vector.tensor_reduce(out=s2, in_=e2e, axis=AxX, op=Alu.add)
                r2 = fb.tile([ROWS, H], f32, tag="r2")
                nc.vector.reciprocal(out=r2, in_=s2)
                p2_t = fb.tile([ROWS, H, T], bf16, tag="p2_t")
                nc.vector.tensor_tensor(
                    out=p2_t, in0=e2e,
                    in1=r2.rearrange("p (h o) -> p h o", o=1).broadcast_to([ROWS, H, T]),
                    op=Alu.mult)

                o2T = fb.tile([128, 4, 2, 128], fp8, tag="o2T")
                for g in range(2):
                    pstk = fb.tile([128, 128], bf16, tag="pstk")
                    for hh in range(4):
                        ptp = pp_tp.tile([2 * T, 128], bf16, tag="tp")
                        nc.tensor.transpose(
                            ptp,
                            p2_t[:, 2 * g * 4 + 2 * hh:2 * g * 4 + 2 * hh + 2, :]
                                .rearrange("p a t -> p (a t)"),
                            k["identb"])
                        nc.vector.tensor_copy(out=pstk[hh * 32:(hh + 1) * 32, :],
                                              in_=ptp)
                    for c2 in range(4):
                        ops = pp_sc.tile([128, 128], f32, tag="sc")
                        nc.tensor.matmul(ops, v2bd[:, g, c2 * 128:(c2 + 1) * 128],
                                         pstk, start=True, stop=True)
                        cc = g * 4 + c2
                        nc.vector.tensor_copy(out=o2T[:, cc // 2, cc % 2, :],
                                              in_=ops)

                # ---- proj_keyboard + residual -> out (reuse x_s storage) ----
                for n in range(3):
                    sl = slice(n * 512, (n + 1) * 512)
                    psp = pp_mm.tile([ROWS, 512], f32, tag="mm")
                    dr_gemm(psp, o2T, k["wpk"], sl)
                    nc.vector.tensor_tensor(out=x_s[:, sl], in0=psp,
                                            in1=xu_s[:, sl], op=Alu.add)
                nc.sync.dma_start(out=out_d[j * ROWS:(j + 1) * ROWS, :], in_=x_s)


_NC_CACHE = {}


def _get_nc(n_tiles=N_TILES, bias_zero=True):
    key = (n_tiles, bias_zero)
    if key not in _NC_CACHE:
        _NC_CACHE[key] = build_nc(n_tiles, bias_zero)
    return _NC_CACHE[key]


def _permute_x(x):
    x3 = np.asarray(x, np.float32).reshape(T, S, C)
    s_idx = np.array([[s0 + u for u in range(SCH)] for s0 in S_STARTS])
    shards = []
    for c in range(8):
        g = x3[:, c * SPC + s_idx, :]          # (T, 25, 8, C)
        g = np.ascontiguousarray(g.transpose(1, 0, 2, 3).reshape(N_TILES * ROWS, C))
        shards.append(g)
    return shards


def _unpermute_out(outs):
    res = np.empty((T, S, C), np.float32)
    j_of_s = np.minimum(np.arange(SPC) // SCH, N_TILES - 1)
    u_of_s = np.arange(SPC) - np.array(S_STARTS)[j_of_s]
    for c in range(8):
        o = np.asarray(outs[c], np.float32).reshape(N_TILES, T, SCH, C)
        res[:, c * SPC:(c + 1) * SPC, :] = o[j_of_s, :, u_of_s, :].transpose(1, 0, 2)
    return res.reshape(1, T * S, C)


def _in_maps(inputs):
    consts, bias_zero = _build_consts(inputs)
    shards = _permute_x(inputs["x"])
    np_dt = {f32: np.float32, bf16: np_bf16, fp8: np_fp8}
    in_maps = []
    for c in range(8):
        # x^T per tile, bf16: [n_tiles, 128, 12, 128]
        sh = shards[c].reshape(N_TILES, ROWS, 12, 128)
        xtt = np.ascontiguousarray(
            sh.astype(np_bf16).transpose(0, 3, 2, 1))
        m = {"xt": shards[c], "xtt": xtt}
        for name, shp, dt in CONST_SPECS:
            m[name] = np.ascontiguousarray(
                np.asarray(consts[name]).astype(np_dt[dt]).reshape(shp))
        in_maps.append(m)
    return in_maps, bias_zero


def run(inputs, trace=False):
    in_maps, bias_zero = _in_maps(inputs)
    nc = _get_nc(bias_zero=bias_zero)
    res = run_bass_kernel_spmd(nc, in_maps, core_ids=list(range(8)),
                               trace=trace)
    return _unpermute_out([r["out"] for r in res.results]), res


def kernel(**inputs):
    out, _ = run(inputs)
    return out


if __name__ == "__main__":
    import time
    t0 = time.time()
    nc = build_nc()
    print("build+compile time:", time.time() - t0)


# revision 20
# speedup vs baseline: 1.4415x; 1.1357x over previous
"""Trainium2 Bass kernel for nn_ActionModule (sparse attention, 8 cores).

Sharding: data-parallel over spatial axis S (1560 = 8 x 195 per core).
Each core processes (T=16 frames x 195 spatial) = 3120 tokens through both
branches; small weights replicated; attention is over T=16 frames only.

Per-core tiling: 25 row-tiles of 128 rows; tile = 16 frames x 8 spatial
positions (row p = t*8 + u). The last tile overlaps the previous one so all
tiles are full 128 rows (overlap rows compute identical values).

v2: fully fused per-tile pipeline (no DRAM round-trip for h or xu), all six
big GEMMs in fp8e4m3 DoubleRow perf mode (2 K-chunks per PE pass), softmax
scale folded into the rope-cos tables, zero biases elided, two SBUF-only
vector ops offloaded to the Pool engine.

Phases (per core):
  P0: keyboard cond MLP -> windows -> k2/v2 (tiny, frame-major, bf16)
  per tile: [x|mouse windows] -> MLP -> LN -> qkv -> RMS+rope -> local
  attention (T=16) -> proj+residual -> key_q -> RMS+rope -> cross attention
  vs k2/v2 -> proj+residual -> out (f32, DRAM)
"""
import sys
sys.path.insert(0, '/opt/trn_rl_repo')

import numpy as np
import ml_dtypes

import bass_rust
import concourse.bass as bass
import concourse.bacc as bacc

# Prefer activation-table set 6 (Ln+Exp+Square together): hide Exp/Ln from
# other sets so the table-load insertion pass never thrashes between the
# exp-only and ln-only tables. The ids passed to walrus stay valid (set 6
# really contains all three); we only constrain the chooser.
_orig_get_tables = bacc.get_activation_tables

def _patched_tables(arch):
    tabs = dict(_orig_get_tables(arch))
    out = {}
    for i, (name, funcs) in enumerate(tabs.items()):
        if name != "natural_log_exp_and_others":
            funcs = {f for f in funcs
                     if f not in (mybir.ActivationFunctionType.Exp,
                                  mybir.ActivationFunctionType.Ln)}
        out[name] = set(funcs)
    return out

bacc.get_activation_tables = _patched_tables
import concourse.bacc as _b2
_b2.get_activation_tables = _patched_tables
import concourse.tile as tile
import concourse.mybir as mybir
from concourse.bass_utils import run_bass_kernel_spmd

f32 = mybir.dt.float32
f32r = mybir.dt.float32r
bf16 = mybir.dt.bfloat16
fp8 = mybir.dt.float8e4
DR = mybir.MatmulPerfMode.DoubleRow
Alu = mybir.AluOpType
Act = mybir.ActivationFunctionType
AxX = mybir.AxisListType.X

np_bf16 = ml_dtypes.bfloat16
np_fp8 = ml_dtypes.float8_e4m3

# dims (hardcoded per spec)
T, TH, TW = 16, 30, 52
S = TH * TW            # 1560
SPC = S // 8           # 195 per core
C = 1536               # IMG
CM = 1024
CK = 1024
HID = 128
HD = 64
H = 16                 # heads
PAD_T = 12             # RATIO*WIN
NF = 61
LOCAL = 6
THETA = 256.0
SCALE = 1.0 / 8.0      # 1/sqrt(64), folded into cgq / cgq2

N_TILES = 25
SCH = 8                # spatial positions per tile
ROWS = T * SCH         # 128
S_STARTS = [min(j * SCH, SPC - SCH) for j in range(N_TILES)]


def _rope_tables():
    t = np.arange(T, dtype=np.float32)
    freqs = 1.0 / (THETA ** (np.arange(0, 8, 2, dtype=np.float32) / 8.0))
    ang = t[:, None] * freqs[None, :]
    cos = np.concatenate([np.cos(ang), np.ones((T, 28), np.float32)], axis=1)
    sin = np.concatenate([np.sin(ang), np.zeros((T, 28), np.float32)], axis=1)
    c_exp = np.repeat(cos, 2, axis=1)   # (16, 64): cos[t, d//2]
    s_exp = np.repeat(sin, 2, axis=1)
    return c_exp, s_exp


def _cg(gain, frame_of_row, scale=1.0):
    """CG (R,64): scale*gain[d]*cos[t(p),d] (shared across heads)."""
    c_exp, _ = _rope_tables()
    return np.ascontiguousarray(
        (scale * gain[None, :] * c_exp[frame_of_row]).astype(np.float32))


def _tan(frame_of_row):
    """tan tables: rope correction on the CG-multiplied values; gains cancel:
      tmp[2i]   = -tan(ang_i) * qcg[2i+1],  tmp[2i+1] = tan(ang_i) * qcg[2i]
    """
    c_exp, s_exp = _rope_tables()
    tg = np.zeros((len(frame_of_row), 8), np.float32)
    for i in range(4):
        tn = s_exp[frame_of_row, 2 * i] / c_exp[frame_of_row, 2 * i]
        tg[:, 2 * i] = -tn
        tg[:, 2 * i + 1] = tn
    return np.tile(tg, (1, 8)).astype(np_bf16)  # (R, 64)


def _pair_w(w, n_out):
    """W [K, N] (K mult of 256) -> fp8 paired [128, K//256, 2, N]."""
    K = w.shape[0]
    assert K % 256 == 0 and w.shape[1] == n_out
    wq = np.clip(w, -240, 240).astype(np_fp8)
    return np.ascontiguousarray(
        wq.reshape(K // 256, 2, 128, n_out).transpose(2, 0, 1, 3))


def _build_consts(inp):
    c = {}
    frame_of_row = (np.arange(ROWS) // SCH).astype(np.int64)
    fr16 = np.arange(T, dtype=np.int64)

    # fold softmax scale into the q-side cos-gain tables
    c["cgq"] = _cg(np.asarray(inp["mq_norm_w"], np.float32), frame_of_row, SCALE)
    c["cgk"] = _cg(np.asarray(inp["mk_norm_w"], np.float32), frame_of_row)
    c["cgq2"] = _cg(np.asarray(inp["kq_norm_w"], np.float32), frame_of_row, SCALE)
    c["cg16"] = _cg(np.asarray(inp["kk_norm_w"], np.float32), fr16)
    c["tanx"] = _tan(frame_of_row)
    c["tanx16"] = _tan(fr16)

    t_p = frame_of_row
    u_p = np.arange(ROWS) % SCH
    same_s = u_p[:, None] == u_p[None, :]
    near_t = np.abs(t_p[:, None] - t_p[None, :]) <= LOCAL
    c["madd"] = np.where(same_s & near_t, 0.0, -1e9).astype(np.float32)
    near2 = np.abs(t_p[:, None] - fr16[None, :]) <= LOCAL
    c["madd2"] = np.where(near2, 0.0, -1e9).astype(np.float32)

    mc = np.asarray(inp["mouse_condition"], np.float32)[0]  # (61, 2)
    gm24t = np.zeros((PAD_T * 2, ROWS), np.float32)
    for w in range(PAD_T):
        src = np.maximum(4 * frame_of_row + w - PAD_T, 0)
        gm24t[2 * w] = mc[src, 0]
        gm24t[2 * w + 1] = mc[src, 1]
    c["gm24t"] = gm24t

    # LN gain fold into qkv weights: W' = diag(g) @ W
    g = np.asarray(inp["mm_ln_g"], np.float32)
    wqkv = g[:, None] * np.asarray(inp["tqkv_w"], np.float32)
    c["wqkv8"] = _pair_w(wqkv[:, :2 * CM], 2 * CM)        # q,k: fp8 DoubleRow
    c["wqkvv"] = np.ascontiguousarray(                    # v: bf16
        wqkv[:, 2 * CM:].astype(np_bf16)
        .reshape(8, 128, CM).transpose(1, 0, 2))

    w1 = np.asarray(inp["mm_w1"], np.float32)
    c["w1"] = np.ascontiguousarray(
        w1[:1536].astype(np_bf16).reshape(12, 128, CM).transpose(1, 0, 2))
    c["w1b"] = w1[1536:1560].copy()                       # (24, CM) f32
    c["w2"] = np.ascontiguousarray(
        np.asarray(inp["mm_w2"], np.float32).astype(np_bf16)
        .reshape(8, 128, CM).transpose(1, 0, 2))
    c["wpm"] = _pair_w(np.asarray(inp["proj_mouse_w"], np.float32), C)
    c["wkq"] = _pair_w(np.asarray(inp["key_q_w"], np.float32), CK)
    c["wpk"] = _pair_w(np.asarray(inp["proj_keyboard_w"], np.float32), C)
    c["wkkv"] = np.asarray(inp["key_kv_w"], np.float32).astype(np_bf16)
    c["kbw1"] = np.asarray(inp["kb_w1"], np.float32)
    c["kbw2"] = np.asarray(inp["kb_w2"], np.float32)
    c["kb1c"] = np.asarray(inp["kb_b1"], np.float32).reshape(HID, 1)
    c["kb2c"] = np.asarray(inp["kb_b2"], np.float32).reshape(HID, 1)
    c["condt"] = np.ascontiguousarray(
        np.asarray(inp["keyboard_condition"], np.float32)[0].T)

    c["identb"] = np.eye(128, dtype=np.float32).astype(np_bf16)
    c["identf8"] = np.eye(128, dtype=np.float32).astype(np_fp8)

    # biases of the two mouse MLP layers + folded LN bias (zero in practice)
    b1 = np.asarray(inp["mm_b1"], np.float32)
    b2 = np.asarray(inp["mm_b2"], np.float32)
    bln = np.asarray(inp["mm_ln_b"], np.float32)
    bq = bln @ np.asarray(inp["tqkv_w"], np.float32)
    c["b1r"] = b1.reshape(1, CM)
    c["b2r"] = b2.reshape(1, CM)
    c["bqkv"] = bq.reshape(1, 3 * CM)
    c["ones"] = np.ones((1, 128), np.float32)
    bias_zero = not (np.any(b1) or np.any(b2) or np.any(bq))
    return c, bias_zero


CONST_SPECS = [
    ("cgq", (ROWS, HD), f32),
    ("cgk", (ROWS, HD), f32),
    ("cgq2", (ROWS, HD), f32),
    ("cg16", (T, HD), f32),
    ("tanx", (ROWS, HD), bf16), ("tanx16", (T, HD), bf16),
    ("madd", (ROWS, ROWS), f32), ("madd2", (ROWS, T), f32),
    ("gm24t", (24, ROWS), f32),
    ("wqkv8", (128, 4, 2, 2 * CM), fp8),
    ("wqkvv", (128, 8, CM), bf16),
    ("w1", (128, 12, CM), bf16), ("w1b", (24, CM), f32),
    ("w2", (128, 8, CM), bf16),
    ("wpm", (128, 4, 2, C), fp8),
    ("wkq", (128, 6, 2, CK), fp8),
    ("wpk", (128, 4, 2, C), fp8),
    ("wkkv", (C, 2 * CK), bf16),
    ("kbw1", (6, HID), f32), ("kbw2", (HID, HID), f32),
    ("kb1c", (HID, 1), f32), ("kb2c", (HID, 1), f32),
    ("condt", (6, NF), f32),
    ("identb", (128, 128), bf16), ("identf8", (128, 128), fp8),
    ("b1r", (1, CM), f32), ("b2r", (1, CM), f32),
    ("bqkv", (1, 3 * CM), f32), ("ones", (1, 128), f32),
]
# loaded inside the program body (not persistent pool)
PHASE_WEIGHTS = {"wkkv", "kbw1", "kbw2", "condt"}
BIAS_CONSTS = {"b1r", "b2r", "bqkv", "ones"}


def build_nc(n_tiles=N_TILES, bias_zero=True):
    nc = bacc.Bacc("TRN2", target_bir_lowering=False, debug=False, num_devices=8)
    xt = nc.dram_tensor("xt", [n_tiles * ROWS, C], f32, kind="ExternalInput").ap()
    xtt = nc.dram_tensor("xtt", [n_tiles, 128, 12, 128], bf16,
                         kind="ExternalInput").ap()
    cst = {}
    for name, shp, dt in CONST_SPECS:
        cst[name] = nc.dram_tensor(name, list(shp), dt, kind="ExternalInput").ap()
    out_d = nc.dram_tensor("out", [n_tiles * ROWS, C], f32, kind="ExternalOutput").ap()
    xu_d = nc.dram_tensor("xu", [n_tiles * ROWS, C], bf16, kind="Internal").ap()
    with tile.TileContext(nc) as tc:
        _prog(nc, tc, xt, xtt, cst, out_d, xu_d, n_tiles, bias_zero)
    nc.compile()
    return nc


def _prog(nc, tc, xt, xtt, cst, out_d, xu_d, n_tiles, bias_zero):
    from contextlib import ExitStack
    with ExitStack() as ctx:
        pers = ctx.enter_context(tc.tile_pool(name="pers", bufs=1))
        pp_mm = ctx.enter_context(tc.tile_pool(name="ppmm", bufs=3, space="PSUM"))
        pp_tp = ctx.enter_context(tc.tile_pool(name="pptp", bufs=2, space="PSUM"))
        pp_sc = ctx.enter_context(tc.tile_pool(name="ppsc", bufs=3, space="PSUM"))

        # ---- persistent consts + weights ----
        k = {}
        for name, shp, dt in CONST_SPECS:
            if name in PHASE_WEIGHTS or name in BIAS_CONSTS \
                    or name in ("gm24t", "w1b", "wqkv8", "wqkvv", "w1", "w2",
                                "wpm", "wkq", "wpk"):
                continue
            t_ = pers.tile(list(shp), dt, tag=name)
            nc.sync.dma_start(out=t_, in_=cst[name])
            k[name] = t_
        gm24t_r = pers.tile([24, ROWS], f32r, tag="gm24t_r")
        nc.sync.dma_start(out=gm24t_r, in_=cst["gm24t"].bitcast(f32r))
        w1b_r = pers.tile([24, CM], f32r, tag="w1b_r")
        nc.sync.dma_start(out=w1b_r, in_=cst["w1b"].bitcast(f32r))
        if not bias_zero:
            for name in BIAS_CONSTS:
                shp = dict((n, s) for n, s, _ in CONST_SPECS)[name]
                t_ = pers.tile(list(shp), f32, tag=name)
                nc.sync.dma_start(out=t_, in_=cst[name])
                k[name] = t_

        eps6 = pers.tile([128, 1], f32, tag="eps6")
        nc.vector.memset(eps6, 1e-6)
        eps5 = pers.tile([128, 1], f32, tag="eps5")
        nc.vector.memset(eps5, 1e-5)

        k2bd = pers.tile([128, 8, 2 * T], bf16, tag="k2bd")
        v2bd = pers.tile([128, 2, 512], bf16, tag="v2bd")

        def trans_copy(src, dst, ident_t, psum_dt, n_chunks, out_view=None):
            """transpose 128-col chunks of src into dst[:, kk, :] (cast on copy)"""
            for kk in range(n_chunks):
                ps = pp_tp.tile([128, 128], psum_dt, tag="tp")
                nc.tensor.transpose(ps, src[:, kk * 128:(kk + 1) * 128], ident_t)
                d = dst[:, kk, :] if out_view is None else out_view(dst, kk)
                nc.vector.tensor_copy(out=d, in_=ps)

        def rms_stage1(scr, q_ps, cg, tg):
            """sq + CG-mult; the only psum readers. Returns (sq, qn, P)."""
            P = q_ps.shape[0]
            HH = 8
            sq = scr.tile([P, 512], bf16, tag="sq" + tg)
            nc.scalar.square(out=sq, in_=q_ps)
            qn = scr.tile([P, 512], bf16, tag="qn" + tg)
            nc.vector.scalar_tensor_tensor(
                out=qn.rearrange("p (h d) -> p h d", h=HH),
                in0=q_ps.rearrange("p (h d) -> p h d", h=HH), scalar=1.0,
                in1=cg.rearrange("p (o d) -> p o d", o=1).broadcast_to([P, HH, HD]),
                op0=Alu.mult, op1=Alu.mult)
            return sq, qn, P

        def rms_stage2(scr, st, tanx, out_half, tg):
            """reduce -> rsqrt -> tan-rope on qn -> apply rstd."""
            sq, qn, P = st
            HH = 8
            qn3 = qn.rearrange("p (h d) -> p h d", h=HH)
            ss = scr.tile([P, HH], f32, tag="ss" + tg)
            nc.vector.tensor_reduce(out=ss, in_=sq.rearrange("p (h d) -> p h d", h=HH),
                                    axis=AxX, op=Alu.add)
            rt = scr.tile([P, HH], f32, tag="rt" + tg)
            nc.scalar.activation(out=rt, in_=ss, func=Act.Ln,
                                 bias=eps6[:P], scale=1.0 / HD)
            rq = scr.tile([P, HH], f32, tag="rq" + tg)
            nc.scalar.activation(out=rq, in_=rt, func=Act.Exp, scale=-0.5)
            # rope correction (first 8 dims/head): tmp = swap(qn) * tanx
            # (even/odd strided 3D ops; >3D DVE ops are rejected)
            tmp = scr.tile([P, HH, 8], bf16, tag="tmp" + tg)

            def _ev(ap, off):
                dims = [list(d) for d in ap.ap]
                step = dims[-1][0]
                nd = dims[:-1] + [[2 * step, 4]]
                return bass_rust.AP(tensor=ap.tensor, offset=ap.offset + off * step,
                                    ap=nd)

            q3r = qn3[:, :, 0:8]
            tmp3 = tmp[:, :, 0:8]
            tx = tanx[:P].rearrange("p (h d) -> p h d", h=HH)
            for off in (0, 1):
                nc.vector.scalar_tensor_tensor(
                    out=_ev(tmp3, off), in0=_ev(q3r, 1 - off), scalar=1.0,
                    in1=_ev(tx, off), op0=Alu.mult, op1=Alu.mult)
            nc.vector.tensor_tensor(out=qn3[:, :, 0:8], in0=qn3[:, :, 0:8], in1=tmp,
                                    op=Alu.add)
            nc.vector.tensor_tensor(
                out=out_half.rearrange("p (h d) -> p h d", h=HH),
                in0=qn3,
                in1=rq.rearrange("p (h o) -> p h o", o=1).broadcast_to([P, HH, HD]),
                op=Alu.mult)

        def rms_rope_half(scr, q_ps, cg, tanx, out_half, tg):
            st = rms_stage1(scr, q_ps, cg, tg)
            rms_stage2(scr, st, tanx, out_half, tg)

        def rms_rope(scr, q_ps2, cg, tanx, out_tile, tg=""):
            if not isinstance(q_ps2, (list, tuple)):
                q_ps2 = [q_ps2[:, 0:512], q_ps2[:, 512:1024]]
            for i in range(2):
                rms_rope_half(scr, q_ps2[i], cg, tanx,
                              out_tile[:, i * 512:(i + 1) * 512], tg + str(i))

        def dr_gemm(ps, actT, w_s, sl, start=True, stop=True):
            npairs = actT.shape[1]
            for p in range(npairs):
                nc.tensor.matmul(ps, actT[:, p], w_s[:, p, :, sl],
                                 start=(start and p == 0),
                                 stop=(stop and p == npairs - 1),
                                 perf_mode=DR)

        # ================= P0: keyboard k2/v2 =================
        with tc.tile_pool(name="p0w", bufs=1) as p0w:
            wkkv_s = p0w.tile([128, 12, 2 * CK], bf16, tag="wkkv")
            for kk in range(12):
                nc.sync.dma_start(out=wkkv_s[:, kk, :],
                                  in_=cst["wkkv"][kk * 128:(kk + 1) * 128, :])
            condt_r = p0w.tile([6, NF], f32, tag="condt_r")
            nc.sync.dma_start(out=condt_r, in_=cst["condt"])
            kbw1_r = p0w.tile([6, HID], f32, tag="kbw1_r")
            nc.sync.dma_start(out=kbw1_r, in_=cst["kbw1"])
            kbw2_r = p0w.tile([HID, HID], f32, tag="kbw2_r")
            nc.sync.dma_start(out=kbw2_r, in_=cst["kbw2"])
            kb1c = p0w.tile([HID, 1], f32, tag="kb1c")
            nc.sync.dma_start(out=kb1c, in_=cst["kb1c"])
            kb2c = p0w.tile([HID, 1], f32, tag="kb2c")
            nc.sync.dma_start(out=kb2c, in_=cst["kb2c"])

            ps0 = pp_mm.tile([HID, NF], f32, tag="mm")
            nc.tensor.matmul(ps0, kbw1_r, condt_r, start=True, stop=True)
            kb1 = p0w.tile([HID, NF], f32, tag="kb1")
            nc.scalar.activation(out=kb1, in_=ps0, func=Act.Silu,
                                 bias=kb1c, scale=1.0)
            ps1 = pp_mm.tile([HID, NF], f32, tag="mm")
            nc.tensor.matmul(ps1, kbw2_r, kb1, start=True, stop=True)
            kb2 = p0w.tile([HID, NF], bf16, tag="kb2")
            nc.vector.tensor_scalar(out=kb2, in0=ps1, scalar1=kb2c, scalar2=None,
                                    op0=Alu.add)
            gkt = p0w.tile([HID, 12, T], bf16, tag="gkt")
            for w in range(12):
                t0 = (12 - w + 3) // 4  # ceil((12-w)/4)
                if t0 > 0:
                    nc.vector.tensor_copy(out=gkt[:, w, 0:t0],
                                          in_=kb2[:, 0:1].broadcast_to([HID, t0]))
                start = 4 * t0 + w - 12
                src = bass_rust.AP(tensor=kb2.tensor, offset=kb2.offset + start,
                                   ap=[list(kb2.ap[0]), [4, T - t0]])
                nc.vector.tensor_copy(out=gkt[:, w, t0:T], in_=src)
            kv_s = p0w.tile([T, 2 * CK], f32, tag="kv_s")
            for n in range(4):
                ps = pp_mm.tile([T, 512], f32, tag="mm")
                for w in range(12):
                    nc.tensor.matmul(ps, gkt[:, w, :],
                                     wkkv_s[:, w, n * 512:(n + 1) * 512],
                                     start=(w == 0), stop=(w == 11))
                nc.vector.tensor_copy(out=kv_s[:, n * 512:(n + 1) * 512], in_=ps)
            k2n = p0w.tile([T, CK], bf16, tag="k2n")
            rms_rope(p0w, kv_s[:, 0:CK], k["cg16"], k["tanx16"], k2n, tg="p0")
            nc.vector.memset(k2bd, 0.0)
            for kk in range(8):
                ps = pp_tp.tile([128, T], bf16, tag="tp")
                nc.tensor.transpose(ps, k2n[:, kk * 128:(kk + 1) * 128],
                                    k["identb"][:T, :T])
                nc.vector.tensor_copy(out=k2bd[0:HD, kk, 0:T], in_=ps[0:HD, :])
                nc.vector.tensor_copy(out=k2bd[HD:128, kk, T:2 * T], in_=ps[HD:128, :])
            nc.vector.memset(v2bd, 0.0)
            v2b_t = p0w.tile([T, CK], bf16, tag="v2b_t")
            nc.vector.tensor_copy(out=v2b_t, in_=kv_s[:, CK:2 * CK])
            for h in range(H):
                g, hh = h // 8, h % 8
                nc.sync.dma_start(
                    out=v2bd[hh * T:(hh + 1) * T, g, hh * HD:(hh + 1) * HD],
                    in_=v2b_t[:, h * HD:(h + 1) * HD])

        h_allT = pers.tile([128, n_tiles, 8, 128], bf16, tag="h_allT")

        # ================= P1: mouse MLP + LN -> h_allT (transposed) ========
        with tc.tile_pool(name="p1w", bufs=1) as p1w, \
             tc.tile_pool(name="p1a", bufs=2) as p1a, \
             tc.tile_pool(name="p1b", bufs=2) as p1b:
            w1_s = p1w.tile([128, 12, CM], bf16, tag="w1")
            for kk in range(12):
                nc.sync.dma_start(out=w1_s[:, kk, :], in_=cst["w1"][:, kk])
            w2_s = p1w.tile([128, 8, CM], bf16, tag="w2")
            for kk in range(8):
                nc.sync.dma_start(out=w2_s[:, kk, :], in_=cst["w2"][:, kk])

            for j in range(n_tiles):
                xT = p1a.tile([128, 12, 128], bf16, tag="xT")
                nc.sync.dma_start(out=xT, in_=xtt[j])
                h1 = p1a.tile([ROWS, CM], bf16, tag="h1")
                for n in range(2):
                    sl = slice(n * 512, (n + 1) * 512)
                    ps1 = pp_mm.tile([ROWS, 512], f32, tag="mm")
                    for kk in range(12):
                        nc.tensor.matmul(ps1, xT[:, kk, :], w1_s[:, kk, sl],
                                         start=(kk == 0), stop=False)
                    nc.tensor.matmul(ps1, gm24t_r, w1b_r[:, sl],
                                     start=False, stop=bias_zero)
                    if not bias_zero:
                        nc.tensor.matmul(ps1, k["ones"].bitcast(f32r),
                                         k["b1r"].bitcast(f32r)[:, sl],
                                         start=False, stop=True)
                    nc.scalar.activation(out=h1[:, sl], in_=ps1,
                                         func=Act.Gelu_apprx_tanh)
                h1T = p1b.tile([128, 8, 128], bf16, tag="h1T")
                trans_copy(h1, h1T, k["identb"], bf16, 8)
                stats = p1b.tile([ROWS, 2, 6], f32, tag="stats")
                ps2h = []
                for n in range(2):
                    sl = slice(n * 512, (n + 1) * 512)
                    ps2 = pp_mm.tile([ROWS, 512], f32, tag="mm")
                    for kk in range(8):
                        nc.tensor.matmul(ps2, h1T[:, kk, :], w2_s[:, kk, sl],
                                         start=(kk == 0), stop=bias_zero and kk == 7)
                    if not bias_zero:
                        nc.tensor.matmul(ps2, k["ones"].bitcast(f32r),
                                         k["b2r"].bitcast(f32r)[:, sl],
                                         start=False, stop=True)
                    nc.vector.bn_stats(out=stats[:, n, :], in_=ps2)
                    ps2h.append(ps2)
                mv = p1b.tile([ROWS, 2], f32, tag="mv")
                nc.vector.bn_aggr(out=mv, in_=stats)
                sd = p1b.tile([ROWS, 1], f32, tag="sd")
                nc.scalar.activation(out=sd, in_=mv[:, 1:2], func=Act.Ln,
                                     bias=eps5, scale=1.0)
                rstd = p1b.tile([ROWS, 1], f32, tag="rstd")
                nc.scalar.activation(out=rstd, in_=sd, func=Act.Exp, scale=-0.5)
                hn = p1a.tile([ROWS, CM], bf16, tag="hn")
                for n in range(2):
                    nc.vector.tensor_scalar(
                        out=hn[:, n * 512:(n + 1) * 512], in0=ps2h[n],
                        scalar1=mv[:, 0:1], scalar2=rstd,
                        op0=Alu.subtract, op1=Alu.mult)
                trans_copy(hn, h_allT, k["identb"], bf16, 8,
                           out_view=lambda d, kk: d[:, j, kk, :])

        # ================= P2: qkv + mouse attention + proj -> xu ===========
        with tc.tile_pool(name="p2w", bufs=1) as p2w, \
             tc.tile_pool(name="p2s", bufs=2) as p2s, \
             tc.tile_pool(name="p2c", bufs=2) as p2c, \
             tc.tile_pool(name="p2a", bufs=2) as p2a, \
             tc.tile_pool(name="p2b", bufs=3) as p2b:
            wqkv8_s = p2w.tile([128, 4, 2, 2 * CM], fp8, tag="wqkv8")
            for p in range(4):
                nc.sync.dma_start(out=wqkv8_s[:, p], in_=cst["wqkv8"][:, p])
            wqkvv_s = p2w.tile([128, 8, CM], bf16, tag="wqkvv")
            for kk in range(8):
                nc.sync.dma_start(out=wqkvv_s[:, kk, :], in_=cst["wqkvv"][:, kk])
            wpm_s = p2w.tile([128, 4, 2, C], fp8, tag="wpm")
            for p in range(4):
                nc.sync.dma_start(out=wpm_s[:, p], in_=cst["wpm"][:, p])

            for j in range(n_tiles):
                x_s = p2a.tile([ROWS, C], f32, tag="x_s2")
                nc.sync.dma_start(out=x_s, in_=xt[j * ROWS:(j + 1) * ROWS, :])
                hTb = h_allT[:, j]
                hT8 = p2c.tile([128, 4, 2, 128], fp8, tag="hT8")
                nc.vector.tensor_copy(
                    out=hT8.rearrange("p a b f -> p (a b f)"),
                    in_=hTb.rearrange("p a f -> p (a f)"))

                qn = p2c.tile([ROWS, CM], bf16, tag="qnb")
                kn = p2c.tile([ROWS, CM], bf16, tag="knb")
                v_s = p2c.tile([ROWS, CM], bf16, tag="v_s")
                halves = [(0, 0, k["cgq"], qn), (0, 1, k["cgq"], qn),
                          (1, 0, k["cgk"], kn), (1, 1, k["cgk"], kn)]
                sts = []
                for i, (part, n, cg, _o) in enumerate(halves):
                    sl = slice(part * CM + n * 512, part * CM + (n + 1) * 512)
                    ps = pp_mm.tile([ROWS, 512], f32, tag="mm")
                    dr_gemm(ps, hT8, wqkv8_s, sl, stop=bias_zero)
                    if not bias_zero:
                        nc.tensor.matmul(ps, k["ones"].bitcast(f32r),
                                         k["bqkv"].bitcast(f32r)[:, sl],
                                         start=False, stop=True)
                    sts.append(rms_stage1(p2s, ps, cg, "h%d" % i))
                for i, (part, n, _c, outt) in enumerate(halves):
                    rms_stage2(p2s, sts[i], k["tanx"],
                               outt[:, n * 512:(n + 1) * 512], "h%d" % i)
                for n in range(2):
                    sl = slice(n * 512, (n + 1) * 512)
                    ps = pp_mm.tile([ROWS, 512], f32, tag="mm")
                    for kk in range(8):
                        nc.tensor.matmul(ps, hTb[:, kk, :], wqkvv_s[:, kk, sl],
                                         start=(kk == 0), stop=bias_zero and kk == 7)
                    if not bias_zero:
                        nc.tensor.matmul(
                            ps, k["ones"].bitcast(f32r),
                            k["bqkv"].bitcast(f32r)[:, 2 * CM + n * 512:
                                                    2 * CM + (n + 1) * 512],
                            start=False, stop=True)
                    nc.vector.tensor_copy(out=v_s[:, n * 512:(n + 1) * 512], in_=ps)

                qT = p2b.tile([128, 8, 128], bf16, tag="qT")
                trans_copy(qn, qT, k["identb"], bf16, 8)
                kT = p2b.tile([128, 8, 128], bf16, tag="kT")
                trans_copy(kn, kT, k["identb"], bf16, 8)

                aoT = p2c.tile([128, 8, 128], fp8, tag="aoT")
                sc_l, es_l, sum_l, pv_l = {}, {}, {}, {}

                def stage_scores(h):
                    n_sl, p_off = h // 2, (h % 2) * HD
                    sc = pp_sc.tile([ROWS, ROWS], f32, tag="sc")
                    nc.tensor.matmul(sc, qT[p_off:p_off + HD, n_sl, :],
                                     kT[p_off:p_off + HD, n_sl, :],
                                     start=True, stop=True)
                    nc.vector.tensor_tensor(out=sc, in0=sc, in1=k["madd"],
                                            op=Alu.add)
                    sc_l[h] = sc

                def stage_exp(h):
                    e_s = p2b.tile([ROWS, ROWS], bf16, tag="e_s")
                    esum = p2b.tile([ROWS, 1], f32, tag="esum")
                    nc.scalar.activation(out=e_s, in_=sc_l.pop(h), func=Act.Exp,
                                         scale=1.0, accum_out=esum)
                    es_l[h], sum_l[h] = e_s, esum

                def stage_out(h):
                    erec = p2b.tile([ROWS, 1], f32, tag="erec")
                    nc.vector.reciprocal(out=erec, in_=sum_l.pop(h))
                    e_c = p2b.tile([ROWS, ROWS], bf16, tag="e_c")
                    nc.scalar.activation(out=e_c, in_=es_l.pop(h),
                                         func=Act.Copy, scale=erec)
                    pt_ps = pp_tp.tile([128, 128], bf16, tag="tp")
                    nc.tensor.transpose(pt_ps, e_c, k["identb"])
                    pt_s = p2b.tile([128, 128], bf16, tag="pt_s")
                    nc.vector.tensor_copy(out=pt_s, in_=pt_ps)
                    if h % 2 == 0:
                        pv_l[h // 2] = pp_sc.tile([128, 128], f32, tag="sc",
                                                  name="pv%d" % (h // 2))
                    pv = pv_l[h // 2]
                    p_off = (h % 2) * HD
                    nc.tensor.matmul(pv[p_off:p_off + HD, :],
                                     v_s[:, h * HD:(h + 1) * HD], pt_s,
                                     start=True, stop=True)
                    if h % 2 == 1:
                        nc.vector.tensor_copy(out=aoT[:, h // 2, :],
                                              in_=pv_l.pop(h // 2))

                for h in range(H + 2):
                    if h < H:
                        stage_scores(h)
                    if 1 <= h <= H:
                        stage_exp(h - 1)
                    if h >= 2:
                        stage_out(h - 2)

                xu_s = p2a.tile([ROWS, C], bf16, tag="xu_s")
                aoTp = aoT.rearrange("p (a b) f -> p a b f", b=2)
                for n in range(3):
                    sl = slice(n * 512, (n + 1) * 512)
                    psp = pp_mm.tile([ROWS, 512], f32, tag="mm")
                    dr_gemm(psp, aoTp, wpm_s, sl)
                    nc.vector.tensor_tensor(out=xu_s[:, sl], in0=psp,
                                            in1=x_s[:, sl], op=Alu.add)
                nc.sync.dma_start(out=xu_d[j * ROWS:(j + 1) * ROWS, :], in_=xu_s)

        # ================= P3: keyboard attention + proj -> out =============
        with tc.tile_pool(name="p3w", bufs=1) as p3w, \
             tc.tile_pool(name="p3s", bufs=2) as p3s, \
             tc.tile_pool(name="p3c", bufs=2) as p3c, \
             tc.tile_pool(name="p3a", bufs=2) as p3a, \
             tc.tile_pool(name="p3b", bufs=3) as p3b:
            wkq_s = p3w.tile([128, 6, 2, CK], fp8, tag="wkq")
            for p in range(6):
                nc.sync.dma_start(out=wkq_s[:, p], in_=cst["wkq"][:, p])
            wpk_s = p3w.tile([128, 4, 2, C], fp8, tag="wpk")
            for p in range(4):
                nc.sync.dma_start(out=wpk_s[:, p], in_=cst["wpk"][:, p])

            for j in range(n_tiles):
                xu_s = p3a.tile([ROWS, C], bf16, tag="xu_s3")
                nc.sync.dma_start(out=xu_s, in_=xu_d[j * ROWS:(j + 1) * ROWS, :])
                xuT = p3c.tile([128, 6, 2, 128], fp8, tag="xuT")
                trans_copy(xu_s, xuT, k["identb"], bf16, 12,
                           out_view=lambda d, kk: d[:, kk // 2, kk % 2, :])
                q2n = p3c.tile([ROWS, CK], bf16, tag="q2n")
                sts2 = []
                for n in range(2):
                    sl = slice(n * 512, (n + 1) * 512)
                    q2_ps = pp_mm.tile([ROWS, 512], f32, tag="mm")
                    dr_gemm(q2_ps, xuT, wkq_s, sl)
                    sts2.append(rms_stage1(p3s, q2_ps, k["cgq2"], "q2%d" % n))
                for n in range(2):
                    rms_stage2(p3s, sts2[n], k["tanx"],
                               q2n[:, n * 512:(n + 1) * 512], "q2%d" % n)
                q2T = p3c.tile([128, 8, 128], bf16, tag="q2T")
                trans_copy(q2n, q2T, k["identb"], bf16, 8)

                sm2 = p3b.tile([ROWS, H, T], f32, tag="sm2")
                for pr in range(H // 2):
                    sc2 = pp_sc.tile([ROWS, 2, T], f32, tag="sc")
                    nc.tensor.matmul(sc2.rearrange("p a t -> p (a t)"),
                                     q2T[:, pr, :], k2bd[:, pr, :],
                                     start=True, stop=True)
                    nc.vector.tensor_tensor(
                        out=sm2[:, 2 * pr:2 * pr + 2, :], in0=sc2,
                        in1=k["madd2"].rearrange("p (o t) -> p o t", o=1)
                            .broadcast_to([ROWS, 2, T]),
                        op=Alu.add)
                e2e = p3b.tile([ROWS, H, T], f32, tag="e2e")
                nc.scalar.activation(out=e2e, in_=sm2, func=Act.Exp)
                s2 = p3b.tile([ROWS, H], f32, tag="s2")
                nc.vector.tensor_reduce(out=s2, in_=e2e, axis=AxX, op=Alu.add)
                r2 = p3b.tile([ROWS, H], f32, tag="r2")
                nc.vector.reciprocal(out=r2, in_=s2)
                p2_t = p3b.tile([ROWS, H, T], bf16, tag="p2_t")
                nc.vector.tensor_tensor(
                    out=p2_t, in0=e2e,
                    in1=r2.rearrange("p (h o) -> p h o", o=1).broadcast_to([ROWS, H, T]),
                    op=Alu.mult)

                o2T = p3b.tile([128, 4, 2, 128], fp8, tag="o2T")
                for g in range(2):
                    pstk = p3b.tile([128, 128], bf16, tag="pstk")
                    for hh in range(4):
                        ptp = pp_tp.tile([2 * T, 128], bf16, tag="tp")
                        nc.tensor.transpose(
                            ptp,
                            p2_t[:, 2 * g * 4 + 2 * hh:2 * g * 4 + 2 * hh + 2, :]
                                .rearrange("p a t -> p (a t)"),
                            k["identb"])
                        nc.vector.tensor_copy(out=pstk[hh * 32:(hh + 1) * 32, :],
                                              in_=ptp)
                    for c2 in range(4):
                        ops = pp_sc.tile([128, 128], f32, tag="sc")
                        nc.tensor.matmul(ops, v2bd[:, g, c2 * 128:(c2 + 1) * 128],
                                         pstk, start=True, stop=True)
                        cc = g * 4 + c2
                        nc.vector.tensor_copy(out=o2T[:, cc // 2, cc % 2, :],
                                              in_=ops)

                fin = p3a.tile([ROWS, C], f32, tag="fin")
                for n in range(3):
                    sl = slice(n * 512, (n + 1) * 512)
                    psp = pp_mm.tile([ROWS, 512], f32, tag="mm")
                    dr_gemm(psp, o2T, wpk_s, sl)
                    nc.vector.tensor_tensor(out=fin[:, sl], in0=psp,
                                            in1=xu_s[:, sl], op=Alu.add)
                nc.sync.dma_start(out=out_d[j * ROWS:(j + 1) * ROWS, :], in_=fin)


_NC_CACHE = {}


def _get_nc(n_tiles=N_TILES, bias_zero=True):
    key = (n_tiles, bias_zero)
    if key not in _NC_CACHE:
        _NC_CACHE[key] = build_nc(n_tiles, bias_zero)
    return _NC_CACHE[key]


def _permute_x(x):
    x3 = np.asarray(x, np.float32).reshape(T, S, C)
    s_idx = np.array([[s0 + u for u in range(SCH)] for s0 in S_STARTS])
    shards = []
    for c in range(8):
        g = x3[:, c * SPC + s_idx, :]          # (T, 25, 8, C)
        g = np.ascontiguousarray(g.transpose(1, 0, 2, 3).reshape(N_TILES * ROWS, C))
        shards.append(g)
    return shards


def _unpermute_out(outs):
    res = np.empty((T, S, C), np.float32)
    j_of_s = np.minimum(np.arange(SPC) // SCH, N_TILES - 1)
    u_of_s = np.arange(SPC) - np.array(S_STARTS)[j_of_s]
    for c in range(8):
        o = np.asarray(outs[c], np.float32).reshape(N_TILES, T, SCH, C)
        res[:, c * SPC:(c + 1) * SPC, :] = o[j_of_s, :, u_of_s, :].transpose(1, 0, 2)
    return res.reshape(1, T * S, C)


def _in_maps(inputs):
    consts, bias_zero = _build_consts(inputs)
    shards = _permute_x(inputs["x"])
    np_dt = {f32: np.float32, bf16: np_bf16, fp8: np_fp8}
    in_maps = []
    for c in range(8):
        # x^T per tile, bf16: [n_tiles, 128, 12, 128]
        sh = shards[c].reshape(N_TILES, ROWS, 12, 128)
        xtt = np.ascontiguousarray(
            sh.astype(np_bf16).transpose(0, 3, 2, 1))
        m = {"xt": shards[c], "xtt": xtt}
        for name, shp, dt in CONST_SPECS:
            m[name] = np.ascontiguousarray(
                np.asarray(consts[name]).astype(np_dt[dt]).reshape(shp))
        in_maps.append(m)
    return in_maps, bias_zero


def run(inputs, trace=False):
    in_maps, bias_zero = _in_maps(inputs)
    nc = _get_nc(bias_zero=bias_zero)
    res = run_bass_kernel_spmd(nc, in_maps, core_ids=list(range(8)),
                               trace=trace)
    return _unpermute_out([r["out"] for r in res.results]), res


def kernel(**inputs):
    out, _ = run(inputs)
    return out


if __name__ == "__main__":
    import time
    t0 = time.time()
    nc = build_nc()
    print("build+compile time:", time.time() - t0)
